# revision 29
# baseline (speedup 1.0000x reference)
"""Trainium2 Bass kernel for nn_Block_42460046688864 (dense transformer block).

Reference math (B=2, T=2048, C=2048, H=16, HD=128):
    n1  = rmsnorm(x) * norm1_w
    qkv = n1 @ attn_w.T ; q,k,v per head ; q,k = rope(q,k) ; phi = elu(.)+1
    w   = (phi_q . phi_k) * scale * tril ; w /= sum(w) ; y = w @ v
    h   = y @ proj_w.T ; x2 = x + h
    ffn = gelu(rmsnorm(x2)*norm2_w @ fc_w.T) @ mlp_proj_w.T ; out = x2 + ffn

Distribution (8 NeuronCores, one NEFF): pure data-parallel over rows.
Each core owns 512 consecutive flattened rows (b-major), computes the
whole block for them, and streams the full weights from HBM (~100MB,
overlapped with ~680us of bf16 matmul).  The causal sum-normalized
elu-kernel attention is computed as chunked linear attention (exactly
equal: the tril mask + positive feature map make masked sum-normalized
scores a prefix recursion; 1/sqrt(HD) and the 1e-8 eps cancel).  The
only cross-core exchange is each core's per-head prefix state
(phi_k^T @ [v|1], 16 x [128,129] bf16 = 528KB), AllGather'd within the
4-core group that shares a batch element, then prefix-masked per core.

Everything else is local: no activation AllGathers/ReduceScatters.

Notes:
  - norm weights are folded into attn_w / fc_w on the host (exact algebra).
  - matmul operands bf16 (fp32 PSUM accumulation); rope/elu elementwise
    runs in bf16 SBUF for the DVE fast modes; residuals stay fp32.
  - TileContext's tail drain is patched to split its semaphore waits:
    this walrus build rejects >2 sync waits on one TPB_CTRL instruction.
"""

from contextlib import ExitStack

import numpy as np
import ml_dtypes

import concourse.bass as bass
import concourse.mybir as mybir
import concourse.tile as tile
from concourse.bass_utils import run_bass_kernel_spmd
from concourse.masks import make_identity
from bass_rust import ScopedClock

F32 = mybir.dt.float32
FP8 = mybir.dt.float8e4
W8SCALE = 64.0
BF16 = mybir.dt.bfloat16
AF = mybir.ActivationFunctionType

N_CORES = 8
GROUP = 4                  # cores per batch element
B, T, C, H, HD = 2, 2048, 2048, 16, 128
HF = HD // 2
R = B * T                  # 4096 flattened rows (b-major)
R_LOC = R // N_CORES       # 512 rows per core
P = 128
N_RT = R_LOC // P          # 4 local row tiles == 4 causal chunks
N_KC = C // P              # 16 contraction tiles over C
FD = 4 * C                 # 8192 mlp hidden
N_FT = FD // P             # 64 hidden tiles
SB = HD + 1                # state cols: [v | 1]
EPS_NORM = 1e-5

_MAX_WAITS = 1  # this walrus build rejects multi-wait instructions


def _split_excess_waits(nc):
    """Move excess semaphore waits onto same-engine NoOps ahead of the op."""
    for fn in nc.m.functions:
        for bb in fn.blocks:
            insts = list(bb.instructions)
            out = []
            for ins in insts:
                si = getattr(ins, "sync_info", None)
                waits = list(si.on_wait) if si and si.on_wait else []
                sem_waits = [w for w in waits if w.sync_type == "semaphore"]
                if len(sem_waits) > _MAX_WAITS:
                    keep = [w for w in waits if w.sync_type != "semaphore"]
                    keep += sem_waits[: _MAX_WAITS - 1] if _MAX_WAITS > 1 else []
                    extra = sem_waits[_MAX_WAITS - 1:] if _MAX_WAITS > 1 else sem_waits
                    for j in range(0, len(extra), _MAX_WAITS):
                        chunk = extra[j:j + _MAX_WAITS]
                        nop = mybir.InstNoOp(
                            name=nc.get_next_instruction_name(), ins=[], outs=[]
                        )
                        nop.engine = ins.engine
                        nop.sync_info = mybir.SyncInfo(on_wait=chunk, on_update=[])
                        out.append(nop)
                    si.on_wait[:] = keep
                out.append(ins)
            if len(out) != len(insts):
                bb.instructions[:] = out


class _TC(tile.TileContext):
    """TileContext whose tail drain splits sem waits one-per-NOP."""

    def schedule_and_allocate(self):
        ret = super().schedule_and_allocate()
        _split_excess_waits(self.nc)
        return ret

    def _drain_and_barrier(self, tick_clock, wait_clock):
        probe = self.nc.sync.nop(nofuse=True, hint="drain_waits")
        wait_clock.add_sem_waits(
            probe.ins, ScopedClock({None: tick_clock.global_clock})
        )
        si = probe.ins.sync_info
        waits = list(si.on_wait) if si and si.on_wait else []
        if len(waits) > 1:
            si.on_wait[:] = waits[:1]
            for w in waits[1:]:
                extra = self.nc.sync.nop(nofuse=True, hint="drain_waits")
                extra.ins.sync_info = mybir.SyncInfo(on_wait=[w], on_update=[])
        self.nc.sync.drain()
        self.nc.all_engine_barrier()
        popped = self.nc._tile_sem_poison_stack.pop()
        assert popped is self._sem_poison
        self.nc.clear_and_free_semaphores(list(self.sems.allocated().values()))
        self.nc.all_engine_barrier()


def build_nc():
    nc = bass.Bass(target_bir_lowering=False)

    x_loc = nc.declare_dram_parameter("x_loc", [R_LOC, C], F32, isOutput=False)
    cosT = nc.declare_dram_parameter("cosT", [HF, R_LOC], BF16, isOutput=False)
    sinT = nc.declare_dram_parameter("sinT", [HF, R_LOC], BF16, isOutput=False)
    maskT = nc.declare_dram_parameter("maskT", [P, P], F32, isOutput=False)
    pmaskp = nc.declare_dram_parameter("pmaskp", [P, GROUP], F32, isOutput=False)
    # attn weight, norm1 folded, transposed; column order [k(16h) | v(16h) | q(16h)]
    attn_w8 = nc.declare_dram_parameter("attn_w8", [N_KC // 2, P, 2 * 3 * C // 1], FP8, isOutput=False)
    projwT = nc.declare_dram_parameter("projwT", [C, C], BF16, isOutput=False)
    fcwT = nc.declare_dram_parameter("fcwT", [C, FD], BF16, isOutput=False)
    mlpw = nc.declare_dram_parameter("mlpw", [FD, C], BF16, isOutput=False)
    out_loc = nc.declare_dram_parameter("out_loc", [R_LOC, C], F32, isOutput=True)

    HH = H // 2
    st_loc = [nc.dram_tensor(f"st_loc{k}", [P, HH * SB], BF16) for k in range(2)]
    st_all = [nc.dram_tensor(f"st_all{k}", [GROUP, P, HH * SB], BF16) for k in range(2)]
    groups = [list(range(GROUP)), list(range(GROUP, 2 * GROUP))]

    with _TC(nc) as tc:
        with ExitStack() as top:
            const = top.enter_context(tc.tile_pool(name="const", bufs=1))
            ident_bf = const.tile([P, P], BF16)
            mask_sb = const.tile([P, P], F32)
            pmask_sb = const.tile([P, GROUP], F32)
            eps_t = const.tile([P, 1], F32)
            cos_sb = const.tile([HF, R_LOC], BF16)
            sin_sb = const.tile([HF, R_LOC], BF16)

            # -------- residents spanning phases A..D (yT) and A..C ---------
            yT_ctx = ExitStack()
            yT_pool = yT_ctx.enter_context(tc.tile_pool(name="yT", bufs=1))
            yT = [yT_pool.tile([P, R_LOC], BF16, name=f"yT{h}") for h in range(H)]

            bc_ctx = ExitStack()
            n1T_pool = bc_ctx.enter_context(tc.tile_pool(name="n1T", bufs=1))
            qkv_pool = bc_ctx.enter_context(tc.tile_pool(name="qkvT", bufs=1))
            vp_pool = bc_ctx.enter_context(tc.tile_pool(name="vp", bufs=1))
            e_pool = bc_ctx.enter_context(tc.tile_pool(name="estate", bufs=1))
            pfx_pool = bc_ctx.enter_context(tc.tile_pool(name="prefix", bufs=1))

            n1T = [
                n1T_pool.tile([P, 2, R_LOC], FP8, name=f"n1T{c}")
                for c in range(N_KC // 2)
            ]
            kTt = [qkv_pool.tile([P, R_LOC], BF16, name=f"kT{h}") for h in range(H)]
            qTt = [qkv_pool.tile([P, R_LOC], BF16, name=f"qT{h}") for h in range(H)]
            vp = [
                [vp_pool.tile([P, SB], BF16, name=f"vp{h}_{i}") for i in range(N_RT)]
                for h in range(H)
            ]
            for h in range(H):
                for i in range(N_RT):
                    nc.vector.memset(vp[h][i][:, HD:SB], 1.0)
            # bf16 exclusive local-state snapshots E_1..E_3 per head + f32 chain
            e_st = [
                [e_pool.tile([P, SB], BF16, name=f"e{h}_{i}") for i in range(3)]
                for h in range(H)
            ]

            # ---------------- phase A: load x, rmsnorm, transpose -> n1T ----
            a_ctx = ExitStack()
            xa_pool = a_ctx.enter_context(tc.tile_pool(name="xa", bufs=1))
            x_tiles = []
            for i in range(N_RT):
                x_t = xa_pool.tile([P, C], F32, name=f"xa{i}")
                eng = nc.scalar if i % 2 else nc.sync
                eng.dma_start(out=x_t[:], in_=x_loc[i * P:(i + 1) * P, :])
                x_tiles.append(x_t)
            make_identity(nc, ident_bf)
            nc.sync.dma_start(out=mask_sb[:], in_=maskT[:, :])
            nc.sync.dma_start(out=pmask_sb[:], in_=pmaskp[:, :])
            nc.vector.memset(eps_t[:], EPS_NORM)
            nc.sync.dma_start(out=cos_sb[:], in_=cosT[:, :])
            nc.sync.dma_start(out=sin_sb[:], in_=sinT[:, :])
            with (
                tc.tile_pool(name="a_st", bufs=1) as a_st,
                tc.tile_pool(name="a_nb", bufs=1) as a_nb,
                tc.tile_pool(name="a_ps", bufs=4, space="PSUM") as a_ps,
            ):
                for i in range(N_RT):
                    x_t = x_tiles[i]
                    sq = a_nb.tile([P, C], F32, name=f"sq{i}", tag="sq", bufs=2)
                    ss = a_st.tile([P, 1], F32, name=f"ss{i}", tag="ss", bufs=2)
                    nc.scalar.activation(sq[:], x_t[:], AF.Square, accum_out=ss[:])
                    rms = a_st.tile([P, 1], F32, name=f"rms{i}", tag="rms", bufs=2)
                    nc.scalar.activation(rms[:], ss[:], AF.Sqrt, bias=eps_t[:], scale=1.0 / C)
                    inv = a_st.tile([P, 1], F32, name=f"inv{i}", tag="inv", bufs=2)
                    nc.vector.reciprocal(inv[:], rms[:])
                    nb = a_nb.tile([P, C], BF16, name=f"n1b{i}", tag="n1b", bufs=2)
                    nc.scalar.activation(nb[:], x_t[:], AF.Copy, scale=inv[:])
                    for j in range(N_KC):
                        ps = a_ps.tile([P, P], BF16, name=f"atr{i}_{j}", tag="atr")
                        nc.tensor.transpose(ps[:], nb[:, j * P:(j + 1) * P], ident_bf[:])
                        nc.scalar.copy(
                            n1T[j // 2][:, j % 2, i * P:(i + 1) * P], ps[:]
                        )
            a_ctx.close()

            # ---------------- phase B: qkv + rope/elu + states + AllGather --
            def rope_elu(ps, dst, rp):
                """psum [P,512] (hd x t) -> dst bf16 [P,512] = elu(rope(.))+1.

                The two psum halves are evicted into separate base-0 tiles:
                DVE tensor-tensor requires equal base partitions for SBUF
                operands, and base-0 keeps the all-bf16 2x path legal.
                """
                qe1 = rp.tile([HF, R_LOC], BF16, name="qe1", tag="qe1", bufs=3)
                nc.scalar.activation(qe1[:], ps[0:HF, :], AF.Copy, scale=1.0 / W8SCALE)
                qe2 = rp.tile([HF, R_LOC], BF16, name="qe2", tag="qe2", bufs=3)
                nc.scalar.activation(qe2[:], ps[HF:P, :], AF.Copy, scale=1.0 / W8SCALE)
                ro = rp.tile([P, R_LOC], BF16, name="ro", tag="ro", bufs=2)
                s1 = rp.tile([HF, R_LOC], BF16, name="s1", tag="s1", bufs=2)
                s2 = rp.tile([HF, R_LOC], BF16, name="s2", tag="s2", bufs=2)
                nc.vector.tensor_mul(s1[:], qe1[:], cos_sb[:])
                nc.vector.tensor_mul(s2[:], qe2[:], sin_sb[:])
                nc.vector.tensor_sub(ro[0:HF, :], s1[:], s2[:])
                s3 = rp.tile([HF, R_LOC], BF16, name="s3", tag="s3", bufs=2)
                s4 = rp.tile([HF, R_LOC], BF16, name="s4", tag="s4", bufs=2)
                nc.vector.tensor_mul(s3[:], qe1[:], sin_sb[:])
                nc.vector.tensor_mul(s4[:], qe2[:], cos_sb[:])
                nc.vector.tensor_add(ro[HF:P, :], s3[:], s4[:])
                # phi = relu(ro) + exp(min(ro, 0))
                rl = rp.tile([P, R_LOC], BF16, name="rl", tag="rl", bufs=2)
                nc.vector.tensor_scalar_max(rl[:], ro[:], 0.0)
                dm = rp.tile([P, R_LOC], BF16, name="dm", tag="dm", bufs=2)
                nc.vector.tensor_scalar_min(dm[:], ro[:], 0.0)
                ex = rp.tile([P, R_LOC], BF16, name="ex", tag="ex", bufs=2)
                nc.scalar.activation(ex[:], dm[:], AF.Exp)
                nc.vector.tensor_add(dst[:], rl[:], ex[:])

            with (
                tc.tile_pool(name="b_aw", bufs=1) as b_aw,
                tc.tile_pool(name="b_rp", bufs=1) as b_rp,
                tc.tile_pool(name="b_vt", bufs=4) as b_vt,
                tc.tile_pool(name="b_kp", bufs=1) as b_kp,
                tc.tile_pool(name="b_stb", bufs=4) as b_stb,
                tc.tile_pool(name="b_ps", bufs=3, space="PSUM") as b_ps,
                tc.tile_pool(name="b_sd", bufs=3, space="PSUM") as b_sd,
                tc.tile_pool(name="b_tr", bufs=2, space="PSUM") as b_tr,
            ):
                kp = [
                    [b_kp.tile([P, P], BF16, name=f"kp{h}_{i}") for i in range(N_RT)]
                    for h in range(H)
                ]

                def state_chain(h):
                    # prefix sums as independent PSUM accumulation groups: no
                    # cross-engine serial chain, just 10 tiny matmuls
                    for n in range(N_RT):  # E_{n+1} = sum_{i<=n} Sd_i
                        sd = b_sd.tile([P, SB], F32, name=f"sd{h}_{n}", tag="sd")
                        for i in range(n + 1):
                            nc.tensor.matmul(
                                sd[:], kp[h][i][:], vp[h][i][:],
                                start=(i == 0), stop=(i == n),
                            )
                        if n < 3:
                            nc.scalar.copy(e_st[h][n][:], sd[:])
                        else:
                            tb = b_stb.tile([P, SB], BF16, name=f"tb{h}", tag="tb")
                            nc.scalar.copy(tb[:], sd[:])
                            nc.scalar.dma_start(
                                out=st_loc[h // HH][
                                    :, (h % HH) * SB:(h % HH + 1) * SB
                                ],
                                in_=tb[:],
                            )

                def qkv_block2(og2):
                    # one contiguous DMA per (k-pair, og-pair): host layout
                    # attn_w8[kq, p, og2*2048 + half*1024 + col]
                    aw = []
                    for kq in range(N_KC // 2):
                        w_t = b_aw.tile(
                            [P, 2, 1024], FP8, name=f"aw{kq}_{og2}", tag=f"aw{kq}",
                            bufs=2,
                        )
                        eng = nc.gpsimd if kq % 2 else nc.sync
                        eng.dma_start(
                            out=w_t[:],
                            in_=attn_w8[kq, :, og2 * 2048:(og2 + 1) * 2048],
                        )
                        aw.append(w_t)
                    for ot in range(8):
                        j = og2 * 8 + ot
                        ps = b_ps.tile([P, R_LOC], F32, name=f"qkvp{j}", tag="qkvp")
                        for kq in range(N_KC // 2):
                            nc.tensor.matmul(
                                ps[:],
                                aw[kq][:, :, ot * P:(ot + 1) * P],
                                n1T[kq][:],
                                start=(kq == 0),
                                stop=(kq == N_KC // 2 - 1),
                                perf_mode=mybir.MatmulPerfMode.DoubleRow,
                            )
                        if og2 < 4:  # interleaved [k_h | v_h] pairs
                            h = j // 2
                            if j % 2 == 0:
                                rope_elu(ps, kTt[h], b_rp)
                                for i in range(N_RT):
                                    ktr = b_tr.tile(
                                        [P, P], BF16, name=f"ktr{h}_{i}", tag="tr"
                                    )
                                    nc.tensor.transpose(
                                        ktr[:], kTt[h][:, i * P:(i + 1) * P],
                                        ident_bf[:],
                                    )
                                    nc.scalar.copy(kp[h][i][:], ktr[:])
                            else:
                                vt = b_vt.tile([P, R_LOC], BF16, name=f"vT{h}", tag="vT")
                                nc.scalar.activation(
                                    vt[:], ps[:], AF.Copy, scale=1.0 / W8SCALE
                                )
                                for i in range(N_RT):
                                    vtr = b_tr.tile(
                                        [P, P], BF16, name=f"vtr{h}_{i}", tag="tr"
                                    )
                                    nc.tensor.transpose(
                                        vtr[:], vt[:, i * P:(i + 1) * P], ident_bf[:]
                                    )
                                    nc.scalar.copy(vp[h][i][:, 0:HD], vtr[:])
                        else:  # q head j-32
                            h = j - 2 * H
                            rope_elu(ps, qTt[h], b_rp)

                # interleaved k/v column groups; each head's state chain is
                # emitted one og later so PE never waits on the transposes.
                # The state gather is split in half so the first collective
                # flies while the second half of k/v is still computing.
                def gather(k):
                    nc.gpsimd.collective_compute(
                        "AllGather",
                        mybir.AluOpType.bypass,
                        ins=[st_loc[k].ap().opt()],
                        outs=[st_all[k].ap().opt()],
                        replica_groups=groups,
                    )

                for og2 in range(4):
                    qkv_block2(og2)
                    for hh in range(4 * (og2 - 1), 4 * og2):
                        if hh >= 0:
                            state_chain(hh)
                    if og2 == 2:
                        gather(0)
                for hh in range(12, 16):
                    state_chain(hh)
                gather(1)

                # q column groups (overlap the collective transfers)
                for og2 in range(4, 6):
                    qkv_block2(og2)

            # ---------------- phase C: prefix + attention ----------------
            with (
                tc.tile_pool(name="c_g", bufs=1) as c_g,
                tc.tile_pool(name="c_cmb", bufs=8) as c_cmb,
                tc.tile_pool(name="c_am", bufs=4) as c_am,
                tc.tile_pool(name="c_sm", bufs=1) as c_sm,
                tc.tile_pool(name="c_aps", bufs=2, space="PSUM") as c_aps,
                tc.tile_pool(name="c_yps", bufs=3, space="PSUM") as c_yps,
                tc.tile_pool(name="c_ytr", bufs=2, space="PSUM") as c_ytr,
            ):
                pwt0 = []
                for h in range(H):
                    w_t = yT_pool.tile([P, 512], BF16, name=f"pw0_{h}")
                    eng = nc.gpsimd if h % 2 else nc.sync
                    eng.dma_start(out=w_t[:], in_=projwT[h * P:(h + 1) * P, 0:512])
                    pwt0.append(w_t)
                pfx = [
                    pfx_pool.tile([P, HH * SB], BF16, name=f"prefix{k}")
                    for k in range(2)
                ]
                gtmp = [
                    pfx_pool.tile([P, HH * SB], BF16, name=f"gtmp{k}")
                    for k in range(2)
                ]
                for k in range(2):
                    for j in range(GROUP):
                        g_t = c_g.tile(
                            [P, HH * SB], BF16, name=f"g{k}_{j}", tag="g", bufs=3
                        )
                        nc.sync.dma_start(out=g_t[:], in_=st_all[k][j])
                        if j == 0:
                            nc.gpsimd.tensor_scalar_mul(
                                pfx[k][:], g_t[:], pmask_sb[:, 0:1]
                            )
                        else:
                            nc.gpsimd.tensor_scalar_mul(
                                gtmp[k][:], g_t[:], pmask_sb[:, j:j + 1]
                            )
                            nc.gpsimd.tensor_add(
                                pfx[k][:], pfx[k][:], gtmp[k][:]
                            )

                for h in range(H):
                    pfx_h = pfx[h // HH]
                    hsl = slice((h % HH) * SB, (h % HH + 1) * SB)
                    for i in range(N_RT):
                        isl = slice(i * P, (i + 1) * P)
                        cmb = c_cmb.tile([P, SB], BF16, name=f"cmb{h}_{i}", tag="cmb")
                        if i == 0:
                            nc.gpsimd.tensor_copy(cmb[:], pfx_h[:, hsl])
                        else:
                            nc.vector.tensor_add(
                                cmb[:], pfx_h[:, hsl], e_st[h][i - 1][:]
                            )
                        a_ps = c_aps.tile([P, P], F32, name=f"a{h}_{i}", tag="a")
                        nc.tensor.matmul(
                            a_ps[:], kTt[h][:, isl], qTt[h][:, isl],
                            start=True, stop=True,
                        )
                        am = c_am.tile([P, P], BF16, name=f"am{h}_{i}", tag="am")
                        nc.vector.tensor_mul(am[:], a_ps[:], mask_sb[:])
                        y_ps = c_yps.tile([P, SB], F32, name=f"y{h}_{i}", tag="y")
                        nc.tensor.matmul(
                            y_ps[:], qTt[h][:, isl], cmb[:], start=True, stop=False
                        )
                        nc.tensor.matmul(
                            y_ps[:], am[:], vp[h][i][:], start=False, stop=True
                        )
                        rec = c_sm.tile([P, 1], F32, name=f"rec{h}_{i}", tag="rec", bufs=4)
                        nc.vector.reciprocal(rec[:], y_ps[:, HD:SB])
                        y_bf = c_sm.tile([P, HD], BF16, name=f"yb{h}_{i}", tag="yb", bufs=4)
                        nc.scalar.activation(y_bf[:], y_ps[:, 0:HD], AF.Copy, scale=rec[:])
                        ytr = c_ytr.tile([P, P], BF16, name=f"ytr{h}_{i}", tag="ytr")
                        nc.tensor.transpose(ytr[:], y_bf[:], ident_bf[:])
                        if h % 2:
                            nc.vector.tensor_copy(yT[h][:, isl], ytr[:])
                        else:
                            nc.scalar.copy(yT[h][:, isl], ytr[:])

            bc_ctx.close()  # free qkv residents for the MLP phases

            # ---------------- phase D: proj + residual + rmsnorm2 ----------
            # x2 spans D..F; n2T and gT span D..F region
            df_ctx = ExitStack()
            x2_pool = df_ctx.enter_context(tc.tile_pool(name="x2", bufs=1))
            n2T_pool = df_ctx.enter_context(tc.tile_pool(name="n2T", bufs=1))
            x2 = [x2_pool.tile([P, C], F32, name=f"x2_{i}") for i in range(N_RT)]
            n2T = [n2T_pool.tile([P, R_LOC], BF16, name=f"n2T{c}") for c in range(N_KC)]

            with (
                tc.tile_pool(name="d_pw", bufs=1) as d_pw,
                tc.tile_pool(name="d_xr", bufs=1) as d_xr,
                tc.tile_pool(name="d_st", bufs=1) as d_st,
                tc.tile_pool(name="d_nb", bufs=1) as d_nb,
                tc.tile_pool(name="d_nb2", bufs=1) as d_nb2,
                tc.tile_pool(name="d_ps", bufs=3, space="PSUM") as d_ps,
                tc.tile_pool(name="d_tps", bufs=4, space="PSUM") as d_tps,
            ):
                def pw_loads(cb):
                    csl = slice(cb * 512, (cb + 1) * 512)
                    pwt = []
                    for h in range(H):
                        w_t = d_pw.tile(
                            [P, 512], BF16, name=f"pw{h}_{cb}", tag=f"pw{h}", bufs=2
                        )
                        eng = nc.gpsimd if (cb >= 2 and h % 2) else nc.sync
                        eng.dma_start(
                            out=w_t[:], in_=projwT[h * P:(h + 1) * P, csl]
                        )
                        pwt.append(w_t)
                    return pwt

                ssp = [[None] * 4 for _ in range(N_RT)]
                pw_next = pwt0
                x_re = []
                for i in range(N_RT):
                    x_t = d_xr.tile([P, C], F32, name=f"xr{i}")
                    eng = nc.scalar if i % 2 else nc.sync
                    eng.dma_start(out=x_t[:], in_=x_loc[i * P:(i + 1) * P, :])
                    x_re.append(x_t)
                for cb in range(4):
                    csl = slice(cb * 512, (cb + 1) * 512)
                    pwt = pw_next
                    if cb < 3:
                        pw_next = pw_loads(cb + 1)
                    for rt in range(N_RT):
                        rsl = slice(rt * P, (rt + 1) * P)
                        ps = d_ps.tile([P, 512], F32, name=f"hp{rt}_{cb}", tag="hp")
                        for h in range(H):
                            nc.tensor.matmul(
                                ps[:],
                                yT[h][:, rsl],
                                pwt[h][:],
                                start=(h == 0),
                                stop=(h == H - 1),
                            )
                        nc.vector.tensor_add(x2[rt][:, csl], x_re[rt][:, csl], ps[:])
                        sqp = d_nb.tile(
                            [P, 512], F32, name=f"dsq{rt}_{cb}", tag="dsq", bufs=3
                        )
                        ssp[rt][cb] = d_st.tile(
                            [P, 1], F32, name=f"dssp{rt}_{cb}", tag=f"ssp{rt}_{cb}"
                        )
                        nc.scalar.activation(
                            sqp[:], x2[rt][:, csl], AF.Square,
                            accum_out=ssp[rt][cb][:],
                        )
                for rt in range(N_RT):
                    s01 = d_st.tile([P, 1], F32, name=f"s01_{rt}", tag="s01", bufs=2)
                    nc.vector.tensor_add(s01[:], ssp[rt][0][:], ssp[rt][1][:])
                    s23 = d_st.tile([P, 1], F32, name=f"s23_{rt}", tag="s23", bufs=2)
                    nc.vector.tensor_add(s23[:], ssp[rt][2][:], ssp[rt][3][:])
                    ss = d_st.tile([P, 1], F32, name=f"dss{rt}", tag="dss", bufs=2)
                    nc.vector.tensor_add(ss[:], s01[:], s23[:])
                    rms = d_st.tile([P, 1], F32, name=f"drms{rt}", tag="drms", bufs=2)
                    nc.scalar.activation(rms[:], ss[:], AF.Sqrt, bias=eps_t[:], scale=1.0 / C)
                    inv = d_st.tile([P, 1], F32, name=f"dinv{rt}", tag="dinv", bufs=2)
                    nc.vector.reciprocal(inv[:], rms[:])
                    nb = d_nb2.tile([P, C], BF16, name=f"n2b{rt}", tag="n2b", bufs=2)
                    nc.scalar.activation(nb[:], x2[rt][:], AF.Copy, scale=inv[:])
                    for j in range(N_KC):
                        tps = d_tps.tile([P, P], BF16, name=f"dtr{rt}_{j}", tag="dtr")
                        nc.tensor.transpose(tps[:], nb[:, j * P:(j + 1) * P], ident_bf[:])
                        nc.scalar.copy(n2T[j][:, rt * P:(rt + 1) * P], tps[:])

            # ---------------- phase E: fc + gelu -> gT ----------------
            g_ctx = ExitStack()
            g_pool = g_ctx.enter_context(tc.tile_pool(name="gT", bufs=1))
            gT = [g_pool.tile([P, R_LOC], BF16, name=f"g{f}") for f in range(N_FT)]
            with (
                tc.tile_pool(name="e_fw", bufs=1) as e_fw,
                tc.tile_pool(name="e_ps", bufs=3, space="PSUM") as e_ps,
            ):
                for fg in range(16):
                    fw = []
                    for ct in range(N_KC):
                        w_t = e_fw.tile(
                            [P, 512], BF16, name=f"fw{ct}_{fg}", tag=f"fw{ct}", bufs=2
                        )
                        eng = nc.gpsimd if (fg > 0 and ct % 2) else nc.sync
                        eng.dma_start(
                            out=w_t[:],
                            in_=fcwT[ct * P:(ct + 1) * P, fg * 512:(fg + 1) * 512],
                        )
                        fw.append(w_t)
                    for ft in range(4):
                        fi = fg * 4 + ft
                        ps = e_ps.tile([P, R_LOC], F32, name=f"gp{fi}", tag="gp")
                        for ct in range(N_KC):
                            nc.tensor.matmul(
                                ps[:],
                                fw[ct][:, ft * P:(ft + 1) * P],
                                n2T[ct][:],
                                start=(ct == 0),
                                stop=(ct == N_KC - 1),
                            )
                        nc.scalar.activation(gT[fi][:], ps[:], AF.Gelu)

            # ---------------- phase F: mlp_proj + residual -> out ----------
            with (
                tc.tile_pool(name="f_mw", bufs=1) as f_mw,
                tc.tile_pool(name="f_ps", bufs=1, space="PSUM") as f_ps,
                tc.tile_pool(name="f_out", bufs=2) as f_out,
            ):
                for cbh in range(2):
                    csl = slice(cbh * 1024, (cbh + 1) * 1024)
                    pss = [
                        f_ps.tile(
                            [P, 1024], F32, name=f"op{cbh}_{rt}", tag=f"op{rt}", bufs=1
                        )
                        for rt in range(N_RT)
                    ]
                    for fi in range(N_FT):
                        mw_t = f_mw.tile(
                            [P, 1024], BF16, name=f"mw{fi}_{cbh}",
                            tag=f"mw{fi % 12}", bufs=2
                        )
                        eng = nc.gpsimd if fi % 2 else nc.sync
                        eng.dma_start(
                            out=mw_t[:], in_=mlpw[fi * P:(fi + 1) * P, csl]
                        )
                        for rt in range(N_RT):
                            for sub in range(2):
                                ssl = slice(sub * 512, (sub + 1) * 512)
                                nc.tensor.matmul(
                                    pss[rt][:, ssl],
                                    gT[fi][:, rt * P:(rt + 1) * P],
                                    mw_t[:, ssl],
                                    start=(fi == 0),
                                    stop=(fi == N_FT - 1),
                                )
                    for rt in range(N_RT):
                        o_t = f_out.tile([P, 1024], F32, name=f"o{cbh}_{rt}", tag="o")
                        nc.vector.tensor_add(o_t[:], x2[rt][:, csl], pss[rt][:])
                        nc.sync.dma_start(
                            out=out_loc[rt * P:(rt + 1) * P, csl], in_=o_t[:]
                        )
            g_ctx.close()
            df_ctx.close()
            yT_ctx.close()

    return nc


_NC_CACHE = None


def _get_nc():
    global _NC_CACHE
    if _NC_CACHE is None:
        _NC_CACHE = build_nc()
    return _NC_CACHE


def _prep_inputs(x, cos, sin, attention_bias, norm1_w, norm2_w, attn_w, proj_w,
                 fc_w, mlp_proj_w):
    bf = ml_dtypes.bfloat16
    xf = np.ascontiguousarray(np.asarray(x, np.float32).reshape(R, C))
    cosTf = np.asarray(cos, np.float32).T.astype(bf)  # [HF, T]
    sinTf = np.asarray(sin, np.float32).T.astype(bf)
    # mask[s, t] = 1 iff s <= t  (transposed causal tril)
    maskT = np.triu(np.ones((P, P), np.float32))
    w1 = np.asarray(norm1_w, np.float32)
    w2 = np.asarray(norm2_w, np.float32)
    aw = (np.asarray(attn_w, np.float32) * w1[None, :]).reshape(H, 3, HD, C)
    # column order [k0 v0 k1 v1 ... | q heads]; qkv comp order is (q,k,v)
    kv_inter = aw[:, (1, 2)].reshape(2 * H, HD, C)  # k_h, v_h interleaved
    aw_kvq = np.concatenate([kv_inter, aw[:, 0]], axis=0)  # [3H, HD, C]
    f8 = ml_dtypes.float8_e4m3fn
    attn_wTn = (aw_kvq.reshape(3 * C, C).T * np.float32(64.0)).astype(f8)
    # [kq, half, p, og2, col] -> [kq, p, og2, half, col]
    attn_w8n = np.ascontiguousarray(
        attn_wTn.reshape(8, 2, P, 6, 1024).transpose(0, 2, 3, 1, 4).reshape(
            8, P, 12288
        )
    )
    projwTn = np.ascontiguousarray(np.asarray(proj_w, np.float32).T).astype(bf)
    fcwTn = np.ascontiguousarray(
        (np.asarray(fc_w, np.float32) * w2[None, :]).T
    ).astype(bf)
    mlpwn = np.ascontiguousarray(np.asarray(mlp_proj_w, np.float32).T).astype(bf)

    in_maps = []
    for c in range(N_CORES):
        t0 = (c * R_LOC) % T
        pm = np.zeros((P, GROUP), np.float32)
        pm[:, : c % GROUP] = 1.0
        in_maps.append({
            "x_loc": np.ascontiguousarray(xf[R_LOC * c:R_LOC * (c + 1)]),
            "cosT": np.ascontiguousarray(cosTf[:, t0:t0 + R_LOC]),
            "sinT": np.ascontiguousarray(sinTf[:, t0:t0 + R_LOC]),
            "maskT": maskT,
            "pmaskp": pm,
            "attn_w8": attn_w8n,
            "projwT": projwTn,
            "fcwT": fcwTn,
            "mlpw": mlpwn,
        })
    return in_maps


def kernel(**inputs):
    nc = _get_nc()
    in_maps = _prep_inputs(**inputs)
    res = run_bass_kernel_spmd(nc, in_maps, list(range(N_CORES)))
    out = np.concatenate(
        [np.asarray(res.results[c]["out_loc"], np.float32) for c in range(N_CORES)],
        axis=0,
    )
    return out.reshape(B, T, C)


# revision 30
# speedup vs baseline: 1.0153x; 1.0153x over previous
"""Trainium2 Bass kernel for nn_Block_42460046688864 (dense transformer block).

Reference math (B=2, T=2048, C=2048, H=16, HD=128):
    n1  = rmsnorm(x) * norm1_w
    qkv = n1 @ attn_w.T ; q,k,v per head ; q,k = rope(q,k) ; phi = elu(.)+1
    w   = (phi_q . phi_k) * scale * tril ; w /= sum(w) ; y = w @ v
    h   = y @ proj_w.T ; x2 = x + h
    ffn = gelu(rmsnorm(x2)*norm2_w @ fc_w.T) @ mlp_proj_w.T ; out = x2 + ffn

Distribution (8 NeuronCores, one NEFF): pure data-parallel over rows.
Each core owns 512 consecutive flattened rows (b-major), computes the
whole block for them, and streams the full weights from HBM (~100MB,
overlapped with ~680us of bf16 matmul).  The causal sum-normalized
elu-kernel attention is computed as chunked linear attention (exactly
equal: the tril mask + positive feature map make masked sum-normalized
scores a prefix recursion; 1/sqrt(HD) and the 1e-8 eps cancel).  The
only cross-core exchange is each core's per-head prefix state
(phi_k^T @ [v|1], 16 x [128,129] bf16 = 528KB), AllGather'd within the
4-core group that shares a batch element, then prefix-masked per core.

Everything else is local: no activation AllGathers/ReduceScatters.

Notes:
  - norm weights are folded into attn_w / fc_w on the host (exact algebra).
  - matmul operands bf16 (fp32 PSUM accumulation); rope/elu elementwise
    runs in bf16 SBUF for the DVE fast modes; residuals stay fp32.
  - TileContext's tail drain is patched to split its semaphore waits:
    this walrus build rejects >2 sync waits on one TPB_CTRL instruction.
"""

from contextlib import ExitStack

import numpy as np
import ml_dtypes

import concourse.bass as bass
import concourse.mybir as mybir
import concourse.tile as tile
from concourse.bass_utils import run_bass_kernel_spmd
from concourse.masks import make_identity
from bass_rust import ScopedClock

F32 = mybir.dt.float32
FP8 = mybir.dt.float8e4
W8SCALE = 64.0
BF16 = mybir.dt.bfloat16
AF = mybir.ActivationFunctionType

N_CORES = 8
GROUP = 4                  # cores per batch element
B, T, C, H, HD = 2, 2048, 2048, 16, 128
HF = HD // 2
R = B * T                  # 4096 flattened rows (b-major)
R_LOC = R // N_CORES       # 512 rows per core
P = 128
N_RT = R_LOC // P          # 4 local row tiles == 4 causal chunks
N_KC = C // P              # 16 contraction tiles over C
FD = 4 * C                 # 8192 mlp hidden
N_FT = FD // P             # 64 hidden tiles
SB = HD + 1                # state cols: [v | 1]
EPS_NORM = 1e-5

_MAX_WAITS = 1  # this walrus build rejects multi-wait instructions


def _split_excess_waits(nc):
    """Move excess semaphore waits onto same-engine NoOps ahead of the op."""
    for fn in nc.m.functions:
        for bb in fn.blocks:
            insts = list(bb.instructions)
            out = []
            for ins in insts:
                si = getattr(ins, "sync_info", None)
                waits = list(si.on_wait) if si and si.on_wait else []
                sem_waits = [w for w in waits if w.sync_type == "semaphore"]
                if len(sem_waits) > _MAX_WAITS:
                    keep = [w for w in waits if w.sync_type != "semaphore"]
                    keep += sem_waits[: _MAX_WAITS - 1] if _MAX_WAITS > 1 else []
                    extra = sem_waits[_MAX_WAITS - 1:] if _MAX_WAITS > 1 else sem_waits
                    for j in range(0, len(extra), _MAX_WAITS):
                        chunk = extra[j:j + _MAX_WAITS]
                        nop = mybir.InstNoOp(
                            name=nc.get_next_instruction_name(), ins=[], outs=[]
                        )
                        nop.engine = ins.engine
                        nop.sync_info = mybir.SyncInfo(on_wait=chunk, on_update=[])
                        out.append(nop)
                    si.on_wait[:] = keep
                out.append(ins)
            if len(out) != len(insts):
                bb.instructions[:] = out


class _TC(tile.TileContext):
    """TileContext whose tail drain splits sem waits one-per-NOP."""

    def schedule_and_allocate(self):
        ret = super().schedule_and_allocate()
        _split_excess_waits(self.nc)
        return ret

    def _drain_and_barrier(self, tick_clock, wait_clock):
        probe = self.nc.sync.nop(nofuse=True, hint="drain_waits")
        wait_clock.add_sem_waits(
            probe.ins, ScopedClock({None: tick_clock.global_clock})
        )
        si = probe.ins.sync_info
        waits = list(si.on_wait) if si and si.on_wait else []
        if len(waits) > 1:
            si.on_wait[:] = waits[:1]
            for w in waits[1:]:
                extra = self.nc.sync.nop(nofuse=True, hint="drain_waits")
                extra.ins.sync_info = mybir.SyncInfo(on_wait=[w], on_update=[])
        self.nc.sync.drain()
        self.nc.all_engine_barrier()
        popped = self.nc._tile_sem_poison_stack.pop()
        assert popped is self._sem_poison
        self.nc.clear_and_free_semaphores(list(self.sems.allocated().values()))
        self.nc.all_engine_barrier()


def build_nc():
    nc = bass.Bass(target_bir_lowering=False)

    x_loc = nc.declare_dram_parameter("x_loc", [R_LOC, C], F32, isOutput=False)
    cosT = nc.declare_dram_parameter("cosT", [HF, R_LOC], BF16, isOutput=False)
    sinT = nc.declare_dram_parameter("sinT", [HF, R_LOC], BF16, isOutput=False)
    maskT = nc.declare_dram_parameter("maskT", [P, P], F32, isOutput=False)
    pmaskp = nc.declare_dram_parameter("pmaskp", [P, GROUP], F32, isOutput=False)
    # attn weight, norm1 folded, transposed; column order [k(16h) | v(16h) | q(16h)]
    attn_w8 = nc.declare_dram_parameter("attn_w8", [N_KC // 2, P, 2 * 3 * C // 1], FP8, isOutput=False)
    projwT = nc.declare_dram_parameter("projwT", [C, C], BF16, isOutput=False)
    fcwT = nc.declare_dram_parameter("fcwT", [C, FD], BF16, isOutput=False)
    mlpw = nc.declare_dram_parameter("mlpw", [FD, C], BF16, isOutput=False)
    out_loc = nc.declare_dram_parameter("out_loc", [R_LOC, C], F32, isOutput=True)

    HH = H // 2
    st_loc = [nc.dram_tensor(f"st_loc{k}", [P, HH * SB], BF16) for k in range(2)]
    st_all = [nc.dram_tensor(f"st_all{k}", [GROUP, P, HH * SB], BF16) for k in range(2)]
    groups = [list(range(GROUP)), list(range(GROUP, 2 * GROUP))]

    with _TC(nc) as tc:
        with ExitStack() as top:
            const = top.enter_context(tc.tile_pool(name="const", bufs=1))
            ident_bf = const.tile([P, P], BF16)
            mask_sb = const.tile([P, P], F32)
            pmask_sb = const.tile([P, GROUP], F32)
            eps_t = const.tile([P, 1], F32)
            cos_sb = const.tile([HF, R_LOC], BF16)
            sin_sb = const.tile([HF, R_LOC], BF16)

            # -------- residents spanning phases A..D (yT) and A..C ---------
            yT_ctx = ExitStack()
            yT_pool = yT_ctx.enter_context(tc.tile_pool(name="yT", bufs=1))
            yT = [yT_pool.tile([P, R_LOC], BF16, name=f"yT{h}") for h in range(H)]

            bc_ctx = ExitStack()
            n1T_pool = bc_ctx.enter_context(tc.tile_pool(name="n1T", bufs=1))
            qkv_pool = bc_ctx.enter_context(tc.tile_pool(name="qkvT", bufs=1))
            vp_pool = bc_ctx.enter_context(tc.tile_pool(name="vp", bufs=1))
            e_pool = bc_ctx.enter_context(tc.tile_pool(name="estate", bufs=1))
            pfx_pool = bc_ctx.enter_context(tc.tile_pool(name="prefix", bufs=1))

            n1T = [
                n1T_pool.tile([P, 2, R_LOC], FP8, name=f"n1T{c}")
                for c in range(N_KC // 2)
            ]
            kTt = [qkv_pool.tile([P, R_LOC], BF16, name=f"kT{h}") for h in range(H)]
            qTt = [qkv_pool.tile([P, R_LOC], BF16, name=f"qT{h}") for h in range(H)]
            vp = [
                [vp_pool.tile([P, SB], BF16, name=f"vp{h}_{i}") for i in range(N_RT)]
                for h in range(H)
            ]
            for h in range(H):
                for i in range(N_RT):
                    nc.vector.memset(vp[h][i][:, HD:SB], 1.0)
            # bf16 exclusive local-state snapshots E_1..E_3 per head + f32 chain
            e_st = [
                [e_pool.tile([P, SB], BF16, name=f"e{h}_{i}") for i in range(3)]
                for h in range(H)
            ]

            # ---------------- phase A: load x, rmsnorm, transpose -> n1T ----
            a_ctx = ExitStack()
            xa_pool = a_ctx.enter_context(tc.tile_pool(name="xa", bufs=1))
            x_tiles = []
            for i in range(N_RT):
                x_t = xa_pool.tile([P, C], F32, name=f"xa{i}")
                eng = nc.scalar if i % 2 else nc.sync
                eng.dma_start(out=x_t[:], in_=x_loc[i * P:(i + 1) * P, :])
                x_tiles.append(x_t)
            make_identity(nc, ident_bf)
            nc.sync.dma_start(out=mask_sb[:], in_=maskT[:, :])
            nc.sync.dma_start(out=pmask_sb[:], in_=pmaskp[:, :])
            nc.vector.memset(eps_t[:], EPS_NORM)
            nc.sync.dma_start(out=cos_sb[:], in_=cosT[:, :])
            nc.sync.dma_start(out=sin_sb[:], in_=sinT[:, :])
            with (
                tc.tile_pool(name="a_st", bufs=1) as a_st,
                tc.tile_pool(name="a_nb", bufs=1) as a_nb,
                tc.tile_pool(name="a_ps", bufs=4, space="PSUM") as a_ps,
            ):
                for i in range(N_RT):
                    x_t = x_tiles[i]
                    sq = a_nb.tile([P, C], F32, name=f"sq{i}", tag="sq", bufs=2)
                    ss = a_st.tile([P, 1], F32, name=f"ss{i}", tag="ss", bufs=2)
                    nc.scalar.activation(sq[:], x_t[:], AF.Square, accum_out=ss[:])
                    rms = a_st.tile([P, 1], F32, name=f"rms{i}", tag="rms", bufs=2)
                    nc.scalar.activation(rms[:], ss[:], AF.Sqrt, bias=eps_t[:], scale=1.0 / C)
                    inv = a_st.tile([P, 1], F32, name=f"inv{i}", tag="inv", bufs=2)
                    nc.vector.reciprocal(inv[:], rms[:])
                    nb = a_nb.tile([P, C], BF16, name=f"n1b{i}", tag="n1b", bufs=2)
                    nc.scalar.activation(nb[:], x_t[:], AF.Copy, scale=inv[:])
                    for j in range(N_KC):
                        ps = a_ps.tile([P, P], BF16, name=f"atr{i}_{j}", tag="atr")
                        nc.tensor.transpose(ps[:], nb[:, j * P:(j + 1) * P], ident_bf[:])
                        nc.scalar.copy(
                            n1T[j // 2][:, j % 2, i * P:(i + 1) * P], ps[:]
                        )
            a_ctx.close()

            # ---------------- phase B: qkv + rope/elu + states + AllGather --
            def rope_elu(ps, dst, rp):
                """psum [P,512] (hd x t) -> dst bf16 [P,512] = elu(rope(.))+1.

                The two psum halves are evicted into separate base-0 tiles:
                DVE tensor-tensor requires equal base partitions for SBUF
                operands, and base-0 keeps the all-bf16 2x path legal.
                """
                qe1 = rp.tile([HF, R_LOC], BF16, name="qe1", tag="qe1", bufs=3)
                nc.scalar.activation(qe1[:], ps[0:HF, :], AF.Copy, scale=1.0 / W8SCALE)
                qe2 = rp.tile([HF, R_LOC], BF16, name="qe2", tag="qe2", bufs=3)
                nc.scalar.activation(qe2[:], ps[HF:P, :], AF.Copy, scale=1.0 / W8SCALE)
                ro = rp.tile([P, R_LOC], BF16, name="ro", tag="ro", bufs=2)
                s1 = rp.tile([HF, R_LOC], BF16, name="s1", tag="s1", bufs=2)
                s2 = rp.tile([HF, R_LOC], BF16, name="s2", tag="s2", bufs=2)
                nc.vector.tensor_mul(s1[:], qe1[:], cos_sb[:])
                nc.vector.tensor_mul(s2[:], qe2[:], sin_sb[:])
                nc.vector.tensor_sub(ro[0:HF, :], s1[:], s2[:])
                s3 = rp.tile([HF, R_LOC], BF16, name="s3", tag="s3", bufs=2)
                s4 = rp.tile([HF, R_LOC], BF16, name="s4", tag="s4", bufs=2)
                nc.vector.tensor_mul(s3[:], qe1[:], sin_sb[:])
                nc.vector.tensor_mul(s4[:], qe2[:], cos_sb[:])
                nc.vector.tensor_add(ro[HF:P, :], s3[:], s4[:])
                # phi = relu(ro) + exp(min(ro, 0))
                rl = rp.tile([P, R_LOC], BF16, name="rl", tag="rl", bufs=2)
                nc.vector.tensor_scalar_max(rl[:], ro[:], 0.0)
                dm = rp.tile([P, R_LOC], BF16, name="dm", tag="dm", bufs=2)
                nc.vector.tensor_scalar_min(dm[:], ro[:], 0.0)
                ex = rp.tile([P, R_LOC], BF16, name="ex", tag="ex", bufs=2)
                nc.scalar.activation(ex[:], dm[:], AF.Exp)
                nc.vector.tensor_add(dst[:], rl[:], ex[:])

            with (
                tc.tile_pool(name="b_aw", bufs=1) as b_aw,
                tc.tile_pool(name="b_rp", bufs=1) as b_rp,
                tc.tile_pool(name="b_vt", bufs=4) as b_vt,
                tc.tile_pool(name="b_kp", bufs=1) as b_kp,
                tc.tile_pool(name="b_stb", bufs=4) as b_stb,
                tc.tile_pool(name="b_ps", bufs=3, space="PSUM") as b_ps,
                tc.tile_pool(name="b_sd", bufs=3, space="PSUM") as b_sd,
                tc.tile_pool(name="b_tr", bufs=2, space="PSUM") as b_tr,
            ):
                kp = [
                    [b_kp.tile([P, P], BF16, name=f"kp{h}_{i}") for i in range(N_RT)]
                    for h in range(H)
                ]

                def state_chain(h):
                    # prefix sums as independent PSUM accumulation groups: no
                    # cross-engine serial chain, just 10 tiny matmuls
                    for n in range(N_RT):  # E_{n+1} = sum_{i<=n} Sd_i
                        sd = b_sd.tile([P, SB], F32, name=f"sd{h}_{n}", tag="sd")
                        for i in range(n + 1):
                            nc.tensor.matmul(
                                sd[:], kp[h][i][:], vp[h][i][:],
                                start=(i == 0), stop=(i == n),
                            )
                        if n < 3:
                            nc.vector.tensor_copy(e_st[h][n][:], sd[:])
                        else:
                            tb = b_stb.tile([P, SB], BF16, name=f"tb{h}", tag="tb")
                            nc.vector.tensor_copy(tb[:], sd[:])
                            nc.scalar.dma_start(
                                out=st_loc[h // HH][
                                    :, (h % HH) * SB:(h % HH + 1) * SB
                                ],
                                in_=tb[:],
                            )

                def qkv_block2(og2):
                    # one contiguous DMA per (k-pair, og-pair): host layout
                    # attn_w8[kq, p, og2*2048 + half*1024 + col]
                    aw = []
                    for kq in range(N_KC // 2):
                        w_t = b_aw.tile(
                            [P, 2, 1024], FP8, name=f"aw{kq}_{og2}", tag=f"aw{kq}",
                            bufs=2,
                        )
                        eng = nc.gpsimd if kq % 2 else nc.sync
                        eng.dma_start(
                            out=w_t[:],
                            in_=attn_w8[kq, :, og2 * 2048:(og2 + 1) * 2048],
                        )
                        aw.append(w_t)
                    for ot in range(8):
                        j = og2 * 8 + ot
                        ps = b_ps.tile([P, R_LOC], F32, name=f"qkvp{j}", tag="qkvp")
                        for kq in range(N_KC // 2):
                            nc.tensor.matmul(
                                ps[:],
                                aw[kq][:, :, ot * P:(ot + 1) * P],
                                n1T[kq][:],
                                start=(kq == 0),
                                stop=(kq == N_KC // 2 - 1),
                                perf_mode=mybir.MatmulPerfMode.DoubleRow,
                            )
                        if og2 < 4:  # interleaved [k_h | v_h] pairs
                            h = j // 2
                            if j % 2 == 0:
                                rope_elu(ps, kTt[h], b_rp)
                                for i in range(N_RT):
                                    ktr = b_tr.tile(
                                        [P, P], BF16, name=f"ktr{h}_{i}", tag="tr"
                                    )
                                    nc.tensor.transpose(
                                        ktr[:], kTt[h][:, i * P:(i + 1) * P],
                                        ident_bf[:],
                                    )
                                    nc.scalar.copy(kp[h][i][:], ktr[:])
                            else:
                                vt = b_vt.tile([P, R_LOC], BF16, name=f"vT{h}", tag="vT")
                                nc.scalar.activation(
                                    vt[:], ps[:], AF.Copy, scale=1.0 / W8SCALE
                                )
                                for i in range(N_RT):
                                    vtr = b_tr.tile(
                                        [P, P], BF16, name=f"vtr{h}_{i}", tag="tr"
                                    )
                                    nc.tensor.transpose(
                                        vtr[:], vt[:, i * P:(i + 1) * P], ident_bf[:]
                                    )
                                    nc.scalar.copy(vp[h][i][:, 0:HD], vtr[:])
                        else:  # q head j-32
                            h = j - 2 * H
                            rope_elu(ps, qTt[h], b_rp)

                # interleaved k/v column groups; each head's state chain is
                # emitted one og later so PE never waits on the transposes.
                # The state gather is split in half so the first collective
                # flies while the second half of k/v is still computing.
                def gather(k):
                    nc.gpsimd.collective_compute(
                        "AllGather",
                        mybir.AluOpType.bypass,
                        ins=[st_loc[k].ap().opt()],
                        outs=[st_all[k].ap().opt()],
                        replica_groups=groups,
                    )

                for og2 in range(4):
                    qkv_block2(og2)
                    for hh in range(4 * (og2 - 1), 4 * og2):
                        if hh >= 0:
                            state_chain(hh)
                    if og2 == 2:
                        gather(0)
                for hh in range(12, 16):
                    state_chain(hh)
                gather(1)

                # q column groups (overlap the collective transfers)
                for og2 in range(4, 6):
                    qkv_block2(og2)

            # ---------------- phase C: prefix + attention ----------------
            with (
                tc.tile_pool(name="c_g", bufs=1) as c_g,
                tc.tile_pool(name="c_cmb", bufs=8) as c_cmb,
                tc.tile_pool(name="c_am", bufs=4) as c_am,
                tc.tile_pool(name="c_sm", bufs=1) as c_sm,
                tc.tile_pool(name="c_aps", bufs=2, space="PSUM") as c_aps,
                tc.tile_pool(name="c_yps", bufs=3, space="PSUM") as c_yps,
                tc.tile_pool(name="c_ytr", bufs=2, space="PSUM") as c_ytr,
            ):
                pwt0 = []
                for h in range(H):
                    w_t = yT_pool.tile([P, 512], BF16, name=f"pw0_{h}")
                    eng = nc.gpsimd if h % 2 else nc.sync
                    eng.dma_start(out=w_t[:], in_=projwT[h * P:(h + 1) * P, 0:512])
                    pwt0.append(w_t)
                pfx = [
                    pfx_pool.tile([P, HH * SB], BF16, name=f"prefix{k}")
                    for k in range(2)
                ]
                gtmp = [
                    pfx_pool.tile([P, HH * SB], BF16, name=f"gtmp{k}")
                    for k in range(2)
                ]
                for k in range(2):
                    for j in range(GROUP):
                        g_t = c_g.tile(
                            [P, HH * SB], BF16, name=f"g{k}_{j}", tag="g", bufs=3
                        )
                        nc.sync.dma_start(out=g_t[:], in_=st_all[k][j])
                        if j == 0:
                            nc.gpsimd.tensor_scalar_mul(
                                pfx[k][:], g_t[:], pmask_sb[:, 0:1]
                            )
                        else:
                            nc.gpsimd.tensor_scalar_mul(
                                gtmp[k][:], g_t[:], pmask_sb[:, j:j + 1]
                            )
                            nc.gpsimd.tensor_add(
                                pfx[k][:], pfx[k][:], gtmp[k][:]
                            )

                for h in range(H):
                    pfx_h = pfx[h // HH]
                    hsl = slice((h % HH) * SB, (h % HH + 1) * SB)
                    for i in range(N_RT):
                        isl = slice(i * P, (i + 1) * P)
                        cmb = c_cmb.tile([P, SB], BF16, name=f"cmb{h}_{i}", tag="cmb")
                        if i == 0:
                            nc.gpsimd.tensor_copy(cmb[:], pfx_h[:, hsl])
                        else:
                            nc.vector.tensor_add(
                                cmb[:], pfx_h[:, hsl], e_st[h][i - 1][:]
                            )
                        a_ps = c_aps.tile([P, P], F32, name=f"a{h}_{i}", tag="a")
                        nc.tensor.matmul(
                            a_ps[:], kTt[h][:, isl], qTt[h][:, isl],
                            start=True, stop=True,
                        )
                        am = c_am.tile([P, P], BF16, name=f"am{h}_{i}", tag="am")
                        nc.vector.tensor_mul(am[:], a_ps[:], mask_sb[:])
                        y_ps = c_yps.tile([P, SB], F32, name=f"y{h}_{i}", tag="y")
                        nc.tensor.matmul(
                            y_ps[:], qTt[h][:, isl], cmb[:], start=True, stop=False
                        )
                        nc.tensor.matmul(
                            y_ps[:], am[:], vp[h][i][:], start=False, stop=True
                        )
                        rec = c_sm.tile([P, 1], F32, name=f"rec{h}_{i}", tag="rec", bufs=4)
                        nc.vector.reciprocal(rec[:], y_ps[:, HD:SB])
                        y_bf = c_sm.tile([P, HD], BF16, name=f"yb{h}_{i}", tag="yb", bufs=4)
                        nc.scalar.activation(y_bf[:], y_ps[:, 0:HD], AF.Copy, scale=rec[:])
                        ytr = c_ytr.tile([P, P], BF16, name=f"ytr{h}_{i}", tag="ytr")
                        nc.tensor.transpose(ytr[:], y_bf[:], ident_bf[:])
                        if h % 2:
                            nc.vector.tensor_copy(yT[h][:, isl], ytr[:])
                        else:
                            nc.scalar.copy(yT[h][:, isl], ytr[:])

            bc_ctx.close()  # free qkv residents for the MLP phases

            # ---------------- phase D: proj + residual + rmsnorm2 ----------
            # x2 spans D..F; n2T and gT span D..F region
            df_ctx = ExitStack()
            x2_pool = df_ctx.enter_context(tc.tile_pool(name="x2", bufs=1))
            n2T_pool = df_ctx.enter_context(tc.tile_pool(name="n2T", bufs=1))
            x2 = [x2_pool.tile([P, C], F32, name=f"x2_{i}") for i in range(N_RT)]
            n2T = [n2T_pool.tile([P, R_LOC], BF16, name=f"n2T{c}") for c in range(N_KC)]

            with (
                tc.tile_pool(name="d_pw", bufs=1) as d_pw,
                tc.tile_pool(name="d_xr", bufs=1) as d_xr,
                tc.tile_pool(name="d_st", bufs=1) as d_st,
                tc.tile_pool(name="d_nb", bufs=1) as d_nb,
                tc.tile_pool(name="d_nb2", bufs=1) as d_nb2,
                tc.tile_pool(name="d_ps", bufs=3, space="PSUM") as d_ps,
                tc.tile_pool(name="d_tps", bufs=4, space="PSUM") as d_tps,
            ):
                def pw_loads(cb):
                    csl = slice(cb * 512, (cb + 1) * 512)
                    pwt = []
                    for h in range(H):
                        w_t = d_pw.tile(
                            [P, 512], BF16, name=f"pw{h}_{cb}", tag=f"pw{h}", bufs=2
                        )
                        eng = nc.gpsimd if (cb >= 2 and h % 2) else nc.sync
                        eng.dma_start(
                            out=w_t[:], in_=projwT[h * P:(h + 1) * P, csl]
                        )
                        pwt.append(w_t)
                    return pwt

                ssp = [[None] * 4 for _ in range(N_RT)]
                pw_next = pwt0
                x_re = []
                for i in range(N_RT):
                    x_t = d_xr.tile([P, C], F32, name=f"xr{i}")
                    eng = nc.scalar if i % 2 else nc.sync
                    eng.dma_start(out=x_t[:], in_=x_loc[i * P:(i + 1) * P, :])
                    x_re.append(x_t)
                for cb in range(4):
                    csl = slice(cb * 512, (cb + 1) * 512)
                    pwt = pw_next
                    if cb < 3:
                        pw_next = pw_loads(cb + 1)
                    for rt in range(N_RT):
                        rsl = slice(rt * P, (rt + 1) * P)
                        ps = d_ps.tile([P, 512], F32, name=f"hp{rt}_{cb}", tag="hp")
                        for h in range(H):
                            nc.tensor.matmul(
                                ps[:],
                                yT[h][:, rsl],
                                pwt[h][:],
                                start=(h == 0),
                                stop=(h == H - 1),
                            )
                        nc.vector.tensor_add(x2[rt][:, csl], x_re[rt][:, csl], ps[:])
                        sqp = d_nb.tile(
                            [P, 512], F32, name=f"dsq{rt}_{cb}", tag="dsq", bufs=3
                        )
                        ssp[rt][cb] = d_st.tile(
                            [P, 1], F32, name=f"dssp{rt}_{cb}", tag=f"ssp{rt}_{cb}"
                        )
                        nc.scalar.activation(
                            sqp[:], x2[rt][:, csl], AF.Square,
                            accum_out=ssp[rt][cb][:],
                        )
                for rt in range(N_RT):
                    s01 = d_st.tile([P, 1], F32, name=f"s01_{rt}", tag="s01", bufs=2)
                    nc.vector.tensor_add(s01[:], ssp[rt][0][:], ssp[rt][1][:])
                    s23 = d_st.tile([P, 1], F32, name=f"s23_{rt}", tag="s23", bufs=2)
                    nc.vector.tensor_add(s23[:], ssp[rt][2][:], ssp[rt][3][:])
                    ss = d_st.tile([P, 1], F32, name=f"dss{rt}", tag="dss", bufs=2)
                    nc.vector.tensor_add(ss[:], s01[:], s23[:])
                    rms = d_st.tile([P, 1], F32, name=f"drms{rt}", tag="drms", bufs=2)
                    nc.scalar.activation(rms[:], ss[:], AF.Sqrt, bias=eps_t[:], scale=1.0 / C)
                    inv = d_st.tile([P, 1], F32, name=f"dinv{rt}", tag="dinv", bufs=2)
                    nc.vector.reciprocal(inv[:], rms[:])
                    nb = d_nb2.tile([P, C], BF16, name=f"n2b{rt}", tag="n2b", bufs=2)
                    nc.scalar.activation(nb[:], x2[rt][:], AF.Copy, scale=inv[:])
                    for j in range(N_KC):
                        tps = d_tps.tile([P, P], BF16, name=f"dtr{rt}_{j}", tag="dtr")
                        nc.tensor.transpose(tps[:], nb[:, j * P:(j + 1) * P], ident_bf[:])
                        nc.scalar.copy(n2T[j][:, rt * P:(rt + 1) * P], tps[:])

            # ---------------- phase E: fc + gelu -> gT ----------------
            g_ctx = ExitStack()
            g_pool = g_ctx.enter_context(tc.tile_pool(name="gT", bufs=1))
            gT = [g_pool.tile([P, R_LOC], BF16, name=f"g{f}") for f in range(N_FT)]
            with (
                tc.tile_pool(name="e_fw", bufs=1) as e_fw,
                tc.tile_pool(name="e_ps", bufs=3, space="PSUM") as e_ps,
            ):
                for fg in range(16):
                    fw = []
                    for ct in range(N_KC):
                        w_t = e_fw.tile(
                            [P, 512], BF16, name=f"fw{ct}_{fg}", tag=f"fw{ct}", bufs=2
                        )
                        eng = nc.gpsimd if (fg > 0 and ct % 2) else nc.sync
                        eng.dma_start(
                            out=w_t[:],
                            in_=fcwT[ct * P:(ct + 1) * P, fg * 512:(fg + 1) * 512],
                        )
                        fw.append(w_t)
                    for ft in range(4):
                        fi = fg * 4 + ft
                        ps = e_ps.tile([P, R_LOC], F32, name=f"gp{fi}", tag="gp")
                        for ct in range(N_KC):
                            nc.tensor.matmul(
                                ps[:],
                                fw[ct][:, ft * P:(ft + 1) * P],
                                n2T[ct][:],
                                start=(ct == 0),
                                stop=(ct == N_KC - 1),
                            )
                        nc.scalar.activation(gT[fi][:], ps[:], AF.Gelu)

            # ---------------- phase F: mlp_proj + residual -> out ----------
            with (
                tc.tile_pool(name="f_mw", bufs=1) as f_mw,
                tc.tile_pool(name="f_ps", bufs=1, space="PSUM") as f_ps,
                tc.tile_pool(name="f_out", bufs=2) as f_out,
            ):
                for cbh in range(2):
                    csl = slice(cbh * 1024, (cbh + 1) * 1024)
                    pss = [
                        f_ps.tile(
                            [P, 1024], F32, name=f"op{cbh}_{rt}", tag=f"op{rt}", bufs=1
                        )
                        for rt in range(N_RT)
                    ]
                    for fi in range(N_FT):
                        mw_t = f_mw.tile(
                            [P, 1024], BF16, name=f"mw{fi}_{cbh}",
                            tag=f"mw{fi % 12}", bufs=2
                        )
                        eng = nc.gpsimd if fi % 2 else nc.sync
                        eng.dma_start(
                            out=mw_t[:], in_=mlpw[fi * P:(fi + 1) * P, csl]
                        )
                        for rt in range(N_RT):
                            for sub in range(2):
                                ssl = slice(sub * 512, (sub + 1) * 512)
                                nc.tensor.matmul(
                                    pss[rt][:, ssl],
                                    gT[fi][:, rt * P:(rt + 1) * P],
                                    mw_t[:, ssl],
                                    start=(fi == 0),
                                    stop=(fi == N_FT - 1),
                                )
                    for rt in range(N_RT):
                        o_t = f_out.tile([P, 1024], F32, name=f"o{cbh}_{rt}", tag="o")
                        nc.vector.tensor_add(o_t[:], x2[rt][:, csl], pss[rt][:])
                        nc.sync.dma_start(
                            out=out_loc[rt * P:(rt + 1) * P, csl], in_=o_t[:]
                        )
            g_ctx.close()
            df_ctx.close()
            yT_ctx.close()

    return nc


_NC_CACHE = None


def _get_nc():
    global _NC_CACHE
    if _NC_CACHE is None:
        _NC_CACHE = build_nc()
    return _NC_CACHE


def _prep_inputs(x, cos, sin, attention_bias, norm1_w, norm2_w, attn_w, proj_w,
                 fc_w, mlp_proj_w):
    bf = ml_dtypes.bfloat16
    xf = np.ascontiguousarray(np.asarray(x, np.float32).reshape(R, C))
    cosTf = np.asarray(cos, np.float32).T.astype(bf)  # [HF, T]
    sinTf = np.asarray(sin, np.float32).T.astype(bf)
    # mask[s, t] = 1 iff s <= t  (transposed causal tril)
    maskT = np.triu(np.ones((P, P), np.float32))
    w1 = np.asarray(norm1_w, np.float32)
    w2 = np.asarray(norm2_w, np.float32)
    aw = (np.asarray(attn_w, np.float32) * w1[None, :]).reshape(H, 3, HD, C)
    # column order [k0 v0 k1 v1 ... | q heads]; qkv comp order is (q,k,v)
    kv_inter = aw[:, (1, 2)].reshape(2 * H, HD, C)  # k_h, v_h interleaved
    aw_kvq = np.concatenate([kv_inter, aw[:, 0]], axis=0)  # [3H, HD, C]
    f8 = ml_dtypes.float8_e4m3fn
    attn_wTn = (aw_kvq.reshape(3 * C, C).T * np.float32(64.0)).astype(f8)
    # [kq, half, p, og2, col] -> [kq, p, og2, half, col]
    attn_w8n = np.ascontiguousarray(
        attn_wTn.reshape(8, 2, P, 6, 1024).transpose(0, 2, 3, 1, 4).reshape(
            8, P, 12288
        )
    )
    projwTn = np.ascontiguousarray(np.asarray(proj_w, np.float32).T).astype(bf)
    fcwTn = np.ascontiguousarray(
        (np.asarray(fc_w, np.float32) * w2[None, :]).T
    ).astype(bf)
    mlpwn = np.ascontiguousarray(np.asarray(mlp_proj_w, np.float32).T).astype(bf)

    in_maps = []
    for c in range(N_CORES):
        t0 = (c * R_LOC) % T
        pm = np.zeros((P, GROUP), np.float32)
        pm[:, : c % GROUP] = 1.0
        in_maps.append({
            "x_loc": np.ascontiguousarray(xf[R_LOC * c:R_LOC * (c + 1)]),
            "cosT": np.ascontiguousarray(cosTf[:, t0:t0 + R_LOC]),
            "sinT": np.ascontiguousarray(sinTf[:, t0:t0 + R_LOC]),
            "maskT": maskT,
            "pmaskp": pm,
            "attn_w8": attn_w8n,
            "projwT": projwTn,
            "fcwT": fcwTn,
            "mlpw": mlpwn,
        })
    return in_maps


def kernel(**inputs):
    nc = _get_nc()
    in_maps = _prep_inputs(**inputs)
    res = run_bass_kernel_spmd(nc, in_maps, list(range(N_CORES)))
    out = np.concatenate(
        [np.asarray(res.results[c]["out_loc"], np.float32) for c in range(N_CORES)],
        axis=0,
    )
    return out.reshape(B, T, C)


# revision 31
# speedup vs baseline: 1.0307x; 1.0151x over previous
"""Trainium2 Bass kernel for nn_Block_42460046688864 (dense transformer block).

Reference math (B=2, T=2048, C=2048, H=16, HD=128):
    n1  = rmsnorm(x) * norm1_w
    qkv = n1 @ attn_w.T ; q,k,v per head ; q,k = rope(q,k) ; phi = elu(.)+1
    w   = (phi_q . phi_k) * scale * tril ; w /= sum(w) ; y = w @ v
    h   = y @ proj_w.T ; x2 = x + h
    ffn = gelu(rmsnorm(x2)*norm2_w @ fc_w.T) @ mlp_proj_w.T ; out = x2 + ffn

Distribution (8 NeuronCores, one NEFF): pure data-parallel over rows.
Each core owns 512 consecutive flattened rows (b-major), computes the
whole block for them, and streams the full weights from HBM (~100MB,
overlapped with ~680us of bf16 matmul).  The causal sum-normalized
elu-kernel attention is computed as chunked linear attention (exactly
equal: the tril mask + positive feature map make masked sum-normalized
scores a prefix recursion; 1/sqrt(HD) and the 1e-8 eps cancel).  The
only cross-core exchange is each core's per-head prefix state
(phi_k^T @ [v|1], 16 x [128,129] bf16 = 528KB), AllGather'd within the
4-core group that shares a batch element, then prefix-masked per core.

Everything else is local: no activation AllGathers/ReduceScatters.

Notes:
  - norm weights are folded into attn_w / fc_w on the host (exact algebra).
  - matmul operands bf16 (fp32 PSUM accumulation); rope/elu elementwise
    runs in bf16 SBUF for the DVE fast modes; residuals stay fp32.
  - TileContext's tail drain is patched to split its semaphore waits:
    this walrus build rejects >2 sync waits on one TPB_CTRL instruction.
"""

from contextlib import ExitStack

import numpy as np
import ml_dtypes

import concourse.bass as bass
import concourse.mybir as mybir
import concourse.tile as tile
from concourse.bass_utils import run_bass_kernel_spmd
from concourse.masks import make_identity
from bass_rust import ScopedClock

F32 = mybir.dt.float32
FP8 = mybir.dt.float8e4
W8SCALE = 64.0
BF16 = mybir.dt.bfloat16
AF = mybir.ActivationFunctionType

N_CORES = 8
GROUP = 4                  # cores per batch element
B, T, C, H, HD = 2, 2048, 2048, 16, 128
HF = HD // 2
R = B * T                  # 4096 flattened rows (b-major)
R_LOC = R // N_CORES       # 512 rows per core
P = 128
N_RT = R_LOC // P          # 4 local row tiles == 4 causal chunks
N_KC = C // P              # 16 contraction tiles over C
FD = 4 * C                 # 8192 mlp hidden
N_FT = FD // P             # 64 hidden tiles
SB = HD + 1                # state cols: [v | 1]
EPS_NORM = 1e-5

_MAX_WAITS = 1  # this walrus build rejects multi-wait instructions


def _split_excess_waits(nc):
    """Move excess semaphore waits onto same-engine NoOps ahead of the op."""
    for fn in nc.m.functions:
        for bb in fn.blocks:
            insts = list(bb.instructions)
            out = []
            for ins in insts:
                si = getattr(ins, "sync_info", None)
                waits = list(si.on_wait) if si and si.on_wait else []
                sem_waits = [w for w in waits if w.sync_type == "semaphore"]
                if len(sem_waits) > _MAX_WAITS:
                    keep = [w for w in waits if w.sync_type != "semaphore"]
                    keep += sem_waits[: _MAX_WAITS - 1] if _MAX_WAITS > 1 else []
                    extra = sem_waits[_MAX_WAITS - 1:] if _MAX_WAITS > 1 else sem_waits
                    for j in range(0, len(extra), _MAX_WAITS):
                        chunk = extra[j:j + _MAX_WAITS]
                        nop = mybir.InstNoOp(
                            name=nc.get_next_instruction_name(), ins=[], outs=[]
                        )
                        nop.engine = ins.engine
                        nop.sync_info = mybir.SyncInfo(on_wait=chunk, on_update=[])
                        out.append(nop)
                    si.on_wait[:] = keep
                out.append(ins)
            if len(out) != len(insts):
                bb.instructions[:] = out


class _TC(tile.TileContext):
    """TileContext whose tail drain splits sem waits one-per-NOP."""

    def schedule_and_allocate(self):
        ret = super().schedule_and_allocate()
        _split_excess_waits(self.nc)
        return ret

    def _drain_and_barrier(self, tick_clock, wait_clock):
        probe = self.nc.sync.nop(nofuse=True, hint="drain_waits")
        wait_clock.add_sem_waits(
            probe.ins, ScopedClock({None: tick_clock.global_clock})
        )
        si = probe.ins.sync_info
        waits = list(si.on_wait) if si and si.on_wait else []
        if len(waits) > 1:
            si.on_wait[:] = waits[:1]
            for w in waits[1:]:
                extra = self.nc.sync.nop(nofuse=True, hint="drain_waits")
                extra.ins.sync_info = mybir.SyncInfo(on_wait=[w], on_update=[])
        self.nc.sync.drain()
        self.nc.all_engine_barrier()
        popped = self.nc._tile_sem_poison_stack.pop()
        assert popped is self._sem_poison
        self.nc.clear_and_free_semaphores(list(self.sems.allocated().values()))
        self.nc.all_engine_barrier()


def build_nc():
    nc = bass.Bass(target_bir_lowering=False)

    x_loc = nc.declare_dram_parameter("x_loc", [R_LOC, C], F32, isOutput=False)
    cosT = nc.declare_dram_parameter("cosT", [HF, R_LOC], BF16, isOutput=False)
    sinT = nc.declare_dram_parameter("sinT", [HF, R_LOC], BF16, isOutput=False)
    maskT = nc.declare_dram_parameter("maskT", [P, P], F32, isOutput=False)
    pmaskp = nc.declare_dram_parameter("pmaskp", [P, GROUP], F32, isOutput=False)
    # attn weight, norm1 folded, transposed; column order [k(16h) | v(16h) | q(16h)]
    attn_w8 = nc.declare_dram_parameter("attn_w8", [N_KC // 2, P, 2 * 3 * C // 1], FP8, isOutput=False)
    projwT = nc.declare_dram_parameter("projwT", [C, C], BF16, isOutput=False)
    fcwT = nc.declare_dram_parameter("fcwT", [C, FD], BF16, isOutput=False)
    mlpw = nc.declare_dram_parameter("mlpw", [FD, C], BF16, isOutput=False)
    out_loc = nc.declare_dram_parameter("out_loc", [R_LOC, C], F32, isOutput=True)

    HH = H // 2
    st_loc = [nc.dram_tensor(f"st_loc{k}", [P, HH * SB], BF16) for k in range(2)]
    st_all = [nc.dram_tensor(f"st_all{k}", [GROUP, P, HH * SB], BF16) for k in range(2)]
    groups = [list(range(GROUP)), list(range(GROUP, 2 * GROUP))]

    with _TC(nc) as tc:
        with ExitStack() as top:
            const = top.enter_context(tc.tile_pool(name="const", bufs=1))
            ident_bf = const.tile([P, P], BF16)
            mask_sb = const.tile([P, P], F32)
            pmask_sb = const.tile([P, GROUP], F32)
            eps_t = const.tile([P, 1], F32)
            cos_sb = const.tile([HF, R_LOC], BF16)
            sin_sb = const.tile([HF, R_LOC], BF16)

            # -------- residents spanning phases A..D (yT) and A..C ---------
            yT_ctx = ExitStack()
            yT_pool = yT_ctx.enter_context(tc.tile_pool(name="yT", bufs=1))
            yT = [yT_pool.tile([P, R_LOC], BF16, name=f"yT{h}") for h in range(H)]

            bc_ctx = ExitStack()
            n1T_pool = bc_ctx.enter_context(tc.tile_pool(name="n1T", bufs=1))
            qkv_pool = bc_ctx.enter_context(tc.tile_pool(name="qkvT", bufs=1))
            vp_pool = bc_ctx.enter_context(tc.tile_pool(name="vp", bufs=1))
            e_pool = bc_ctx.enter_context(tc.tile_pool(name="estate", bufs=1))
            pfx_pool = bc_ctx.enter_context(tc.tile_pool(name="prefix", bufs=1))

            n1T = [
                n1T_pool.tile([P, 2, R_LOC], FP8, name=f"n1T{c}")
                for c in range(N_KC // 2)
            ]
            kTt = [qkv_pool.tile([P, R_LOC], BF16, name=f"kT{h}") for h in range(H)]
            qTt = [qkv_pool.tile([P, R_LOC], BF16, name=f"qT{h}") for h in range(H)]
            vp = [
                [vp_pool.tile([P, SB], BF16, name=f"vp{h}_{i}") for i in range(N_RT)]
                for h in range(H)
            ]
            for h in range(H):
                for i in range(N_RT):
                    nc.vector.memset(vp[h][i][:, HD:SB], 1.0)
            # bf16 exclusive local-state snapshots E_1..E_3 per head + f32 chain
            e_st = [
                [e_pool.tile([P, SB], BF16, name=f"e{h}_{i}") for i in range(3)]
                for h in range(H)
            ]

            # ---------------- phase A: load x, rmsnorm, transpose -> n1T ----
            a_ctx = ExitStack()
            xa_pool = a_ctx.enter_context(tc.tile_pool(name="xa", bufs=1))
            x_tiles = []
            for i in range(N_RT):
                x_t = xa_pool.tile([P, C], F32, name=f"xa{i}")
                eng = nc.scalar if i % 2 else nc.sync
                eng.dma_start(out=x_t[:], in_=x_loc[i * P:(i + 1) * P, :])
                x_tiles.append(x_t)
            make_identity(nc, ident_bf)
            nc.sync.dma_start(out=mask_sb[:], in_=maskT[:, :])
            nc.sync.dma_start(out=pmask_sb[:], in_=pmaskp[:, :])
            nc.vector.memset(eps_t[:], EPS_NORM)
            nc.sync.dma_start(out=cos_sb[:], in_=cosT[:, :])
            nc.sync.dma_start(out=sin_sb[:], in_=sinT[:, :])
            with (
                tc.tile_pool(name="a_st", bufs=1) as a_st,
                tc.tile_pool(name="a_nb", bufs=1) as a_nb,
                tc.tile_pool(name="a_ps", bufs=4, space="PSUM") as a_ps,
            ):
                for i in range(N_RT):
                    x_t = x_tiles[i]
                    sq = a_nb.tile([P, C], F32, name=f"sq{i}", tag="sq", bufs=2)
                    ss = a_st.tile([P, 1], F32, name=f"ss{i}", tag="ss", bufs=2)
                    nc.scalar.activation(sq[:], x_t[:], AF.Square, accum_out=ss[:])
                    rms = a_st.tile([P, 1], F32, name=f"rms{i}", tag="rms", bufs=2)
                    nc.scalar.activation(rms[:], ss[:], AF.Sqrt, bias=eps_t[:], scale=1.0 / C)
                    inv = a_st.tile([P, 1], F32, name=f"inv{i}", tag="inv", bufs=2)
                    nc.vector.reciprocal(inv[:], rms[:])
                    nb = a_nb.tile([P, C], BF16, name=f"n1b{i}", tag="n1b", bufs=2)
                    nc.scalar.activation(nb[:], x_t[:], AF.Copy, scale=inv[:])
                    for j in range(N_KC):
                        ps = a_ps.tile([P, P], BF16, name=f"atr{i}_{j}", tag="atr")
                        nc.tensor.transpose(ps[:], nb[:, j * P:(j + 1) * P], ident_bf[:])
                        nc.scalar.copy(
                            n1T[j // 2][:, j % 2, i * P:(i + 1) * P], ps[:]
                        )
            a_ctx.close()

            # ---------------- phase B: qkv + rope/elu + states + AllGather --
            def rope_elu(ps, dst, rp):
                """psum [P,512] (hd x t) -> dst bf16 [P,512] = elu(rope(.))+1.

                The two psum halves are evicted into separate base-0 tiles:
                DVE tensor-tensor requires equal base partitions for SBUF
                operands, and base-0 keeps the all-bf16 2x path legal.
                """
                qe1 = rp.tile([HF, R_LOC], BF16, name="qe1", tag="qe1", bufs=3)
                nc.scalar.activation(qe1[:], ps[0:HF, :], AF.Copy, scale=1.0 / W8SCALE)
                qe2 = rp.tile([HF, R_LOC], BF16, name="qe2", tag="qe2", bufs=3)
                nc.scalar.activation(qe2[:], ps[HF:P, :], AF.Copy, scale=1.0 / W8SCALE)
                ro = rp.tile([P, R_LOC], BF16, name="ro", tag="ro", bufs=2)
                s1 = rp.tile([HF, R_LOC], BF16, name="s1", tag="s1", bufs=2)
                s2 = rp.tile([HF, R_LOC], BF16, name="s2", tag="s2", bufs=2)
                nc.vector.tensor_mul(s1[:], qe1[:], cos_sb[:])
                nc.vector.tensor_mul(s2[:], qe2[:], sin_sb[:])
                nc.vector.tensor_sub(ro[0:HF, :], s1[:], s2[:])
                s3 = rp.tile([HF, R_LOC], BF16, name="s3", tag="s3", bufs=2)
                s4 = rp.tile([HF, R_LOC], BF16, name="s4", tag="s4", bufs=2)
                nc.vector.tensor_mul(s3[:], qe1[:], sin_sb[:])
                nc.vector.tensor_mul(s4[:], qe2[:], cos_sb[:])
                nc.vector.tensor_add(ro[HF:P, :], s3[:], s4[:])
                # phi = relu(ro) + exp(min(ro, 0))
                rl = rp.tile([P, R_LOC], BF16, name="rl", tag="rl", bufs=2)
                nc.vector.tensor_scalar_max(rl[:], ro[:], 0.0)
                dm = rp.tile([P, R_LOC], BF16, name="dm", tag="dm", bufs=2)
                nc.vector.tensor_scalar_min(dm[:], ro[:], 0.0)
                ex = rp.tile([P, R_LOC], BF16, name="ex", tag="ex", bufs=2)
                nc.scalar.activation(ex[:], dm[:], AF.Exp)
                nc.vector.tensor_add(dst[:], rl[:], ex[:])

            with (
                tc.tile_pool(name="b_aw", bufs=1) as b_aw,
                tc.tile_pool(name="b_rp", bufs=1) as b_rp,
                tc.tile_pool(name="b_vt", bufs=4) as b_vt,
                tc.tile_pool(name="b_kp", bufs=1) as b_kp,
                tc.tile_pool(name="b_stb", bufs=4) as b_stb,
                tc.tile_pool(name="b_ps", bufs=3, space="PSUM") as b_ps,
                tc.tile_pool(name="b_sd", bufs=3, space="PSUM") as b_sd,
                tc.tile_pool(name="b_tr", bufs=2, space="PSUM") as b_tr,
            ):
                kp = [
                    [b_kp.tile([P, P], BF16, name=f"kp{h}_{i}") for i in range(N_RT)]
                    for h in range(H)
                ]

                def state_chain(h):
                    # prefix sums as independent PSUM accumulation groups: no
                    # cross-engine serial chain, just 10 tiny matmuls
                    for n in range(N_RT):  # E_{n+1} = sum_{i<=n} Sd_i
                        sd = b_sd.tile([P, SB], F32, name=f"sd{h}_{n}", tag="sd")
                        for i in range(n + 1):
                            nc.tensor.matmul(
                                sd[:], kp[h][i][:], vp[h][i][:],
                                start=(i == 0), stop=(i == n),
                            )
                        if n < 3:
                            nc.vector.tensor_copy(e_st[h][n][:], sd[:])
                        else:
                            tb = b_stb.tile([P, SB], BF16, name=f"tb{h}", tag="tb")
                            nc.vector.tensor_copy(tb[:], sd[:])
                            nc.scalar.dma_start(
                                out=st_loc[h // HH][
                                    :, (h % HH) * SB:(h % HH + 1) * SB
                                ],
                                in_=tb[:],
                            )

                def qkv_block2(og2):
                    # one contiguous DMA per (k-pair, og-pair): host layout
                    # attn_w8[kq, p, og2*2048 + half*1024 + col]
                    aw = []
                    for kq in range(N_KC // 2):
                        w_t = b_aw.tile(
                            [P, 2, 1024], FP8, name=f"aw{kq}_{og2}", tag=f"aw{kq}",
                            bufs=2,
                        )
                        eng = nc.gpsimd if kq % 2 else nc.sync
                        eng.dma_start(
                            out=w_t[:],
                            in_=attn_w8[kq, :, og2 * 2048:(og2 + 1) * 2048],
                        )
                        aw.append(w_t)
                    for ot in range(8):
                        j = og2 * 8 + ot
                        ps = b_ps.tile([P, R_LOC], F32, name=f"qkvp{j}", tag="qkvp")
                        for kq in range(N_KC // 2):
                            nc.tensor.matmul(
                                ps[:],
                                aw[kq][:, :, ot * P:(ot + 1) * P],
                                n1T[kq][:],
                                start=(kq == 0),
                                stop=(kq == N_KC // 2 - 1),
                                perf_mode=mybir.MatmulPerfMode.DoubleRow,
                            )
                        if og2 < 4:  # interleaved [k_h | v_h] pairs
                            h = j // 2
                            if j % 2 == 0:
                                rope_elu(ps, kTt[h], b_rp)
                                for i in range(N_RT):
                                    ktr = b_tr.tile(
                                        [P, P], BF16, name=f"ktr{h}_{i}", tag="tr"
                                    )
                                    nc.tensor.transpose(
                                        ktr[:], kTt[h][:, i * P:(i + 1) * P],
                                        ident_bf[:],
                                    )
                                    if i % 2:
                                        nc.vector.tensor_copy(kp[h][i][:], ktr[:])
                                    else:
                                        nc.scalar.copy(kp[h][i][:], ktr[:])
                            else:
                                vt = b_vt.tile([P, R_LOC], BF16, name=f"vT{h}", tag="vT")
                                nc.scalar.activation(
                                    vt[:], ps[:], AF.Copy, scale=1.0 / W8SCALE
                                )
                                for i in range(N_RT):
                                    vtr = b_tr.tile(
                                        [P, P], BF16, name=f"vtr{h}_{i}", tag="tr"
                                    )
                                    nc.tensor.transpose(
                                        vtr[:], vt[:, i * P:(i + 1) * P], ident_bf[:]
                                    )
                                    if i % 2:
                                        nc.vector.tensor_copy(vp[h][i][:, 0:HD], vtr[:])
                                    else:
                                        nc.scalar.copy(vp[h][i][:, 0:HD], vtr[:])
                        else:  # q head j-32
                            h = j - 2 * H
                            rope_elu(ps, qTt[h], b_rp)

                # interleaved k/v column groups; each head's state chain is
                # emitted one og later so PE never waits on the transposes.
                # The state gather is split in half so the first collective
                # flies while the second half of k/v is still computing.
                def gather(k):
                    nc.gpsimd.collective_compute(
                        "AllGather",
                        mybir.AluOpType.bypass,
                        ins=[st_loc[k].ap().opt()],
                        outs=[st_all[k].ap().opt()],
                        replica_groups=groups,
                    )

                for og2 in range(4):
                    qkv_block2(og2)
                    for hh in range(4 * (og2 - 1), 4 * og2):
                        if hh >= 0:
                            state_chain(hh)
                    if og2 == 2:
                        gather(0)
                for hh in range(12, 16):
                    state_chain(hh)
                gather(1)

                # q column groups (overlap the collective transfers)
                for og2 in range(4, 6):
                    qkv_block2(og2)

            # ---------------- phase C: prefix + attention ----------------
            with (
                tc.tile_pool(name="c_g", bufs=1) as c_g,
                tc.tile_pool(name="c_cmb", bufs=8) as c_cmb,
                tc.tile_pool(name="c_am", bufs=4) as c_am,
                tc.tile_pool(name="c_sm", bufs=1) as c_sm,
                tc.tile_pool(name="c_aps", bufs=2, space="PSUM") as c_aps,
                tc.tile_pool(name="c_yps", bufs=3, space="PSUM") as c_yps,
                tc.tile_pool(name="c_ytr", bufs=2, space="PSUM") as c_ytr,
            ):
                pwt0 = []
                for h in range(H):
                    w_t = yT_pool.tile([P, 512], BF16, name=f"pw0_{h}")
                    eng = nc.gpsimd if h % 2 else nc.sync
                    eng.dma_start(out=w_t[:], in_=projwT[h * P:(h + 1) * P, 0:512])
                    pwt0.append(w_t)
                pfx = [
                    pfx_pool.tile([P, HH * SB], BF16, name=f"prefix{k}")
                    for k in range(2)
                ]
                gtmp = [
                    pfx_pool.tile([P, HH * SB], BF16, name=f"gtmp{k}")
                    for k in range(2)
                ]
                for k in range(2):
                    for j in range(GROUP):
                        g_t = c_g.tile(
                            [P, HH * SB], BF16, name=f"g{k}_{j}", tag="g", bufs=3
                        )
                        nc.sync.dma_start(out=g_t[:], in_=st_all[k][j])
                        if j == 0:
                            nc.gpsimd.tensor_scalar_mul(
                                pfx[k][:], g_t[:], pmask_sb[:, 0:1]
                            )
                        else:
                            nc.gpsimd.tensor_scalar_mul(
                                gtmp[k][:], g_t[:], pmask_sb[:, j:j + 1]
                            )
                            nc.gpsimd.tensor_add(
                                pfx[k][:], pfx[k][:], gtmp[k][:]
                            )

                for h in range(H):
                    pfx_h = pfx[h // HH]
                    hsl = slice((h % HH) * SB, (h % HH + 1) * SB)
                    for i in range(N_RT):
                        isl = slice(i * P, (i + 1) * P)
                        cmb = c_cmb.tile([P, SB], BF16, name=f"cmb{h}_{i}", tag="cmb")
                        if i == 0:
                            nc.gpsimd.tensor_copy(cmb[:], pfx_h[:, hsl])
                        else:
                            nc.vector.tensor_add(
                                cmb[:], pfx_h[:, hsl], e_st[h][i - 1][:]
                            )
                        a_ps = c_aps.tile([P, P], F32, name=f"a{h}_{i}", tag="a")
                        nc.tensor.matmul(
                            a_ps[:], kTt[h][:, isl], qTt[h][:, isl],
                            start=True, stop=True,
                        )
                        am = c_am.tile([P, P], BF16, name=f"am{h}_{i}", tag="am")
                        nc.vector.tensor_mul(am[:], a_ps[:], mask_sb[:])
                        y_ps = c_yps.tile([P, SB], F32, name=f"y{h}_{i}", tag="y")
                        nc.tensor.matmul(
                            y_ps[:], qTt[h][:, isl], cmb[:], start=True, stop=False
                        )
                        nc.tensor.matmul(
                            y_ps[:], am[:], vp[h][i][:], start=False, stop=True
                        )
                        rec = c_sm.tile([P, 1], F32, name=f"rec{h}_{i}", tag="rec", bufs=4)
                        nc.vector.reciprocal(rec[:], y_ps[:, HD:SB])
                        y_bf = c_sm.tile([P, HD], BF16, name=f"yb{h}_{i}", tag="yb", bufs=4)
                        nc.scalar.activation(y_bf[:], y_ps[:, 0:HD], AF.Copy, scale=rec[:])
                        ytr = c_ytr.tile([P, P], BF16, name=f"ytr{h}_{i}", tag="ytr")
                        nc.tensor.transpose(ytr[:], y_bf[:], ident_bf[:])
                        if h % 2:
                            nc.vector.tensor_copy(yT[h][:, isl], ytr[:])
                        else:
                            nc.scalar.copy(yT[h][:, isl], ytr[:])

            bc_ctx.close()  # free qkv residents for the MLP phases

            # ---------------- phase D: proj + residual + rmsnorm2 ----------
            # x2 spans D..F; n2T and gT span D..F region
            df_ctx = ExitStack()
            x2_pool = df_ctx.enter_context(tc.tile_pool(name="x2", bufs=1))
            n2T_pool = df_ctx.enter_context(tc.tile_pool(name="n2T", bufs=1))
            x2 = [x2_pool.tile([P, C], F32, name=f"x2_{i}") for i in range(N_RT)]
            n2T = [n2T_pool.tile([P, R_LOC], BF16, name=f"n2T{c}") for c in range(N_KC)]

            with (
                tc.tile_pool(name="d_pw", bufs=1) as d_pw,
                tc.tile_pool(name="d_xr", bufs=1) as d_xr,
                tc.tile_pool(name="d_st", bufs=1) as d_st,
                tc.tile_pool(name="d_nb", bufs=1) as d_nb,
                tc.tile_pool(name="d_nb2", bufs=1) as d_nb2,
                tc.tile_pool(name="d_ps", bufs=3, space="PSUM") as d_ps,
                tc.tile_pool(name="d_tps", bufs=4, space="PSUM") as d_tps,
            ):
                def pw_loads(cb):
                    csl = slice(cb * 512, (cb + 1) * 512)
                    pwt = []
                    for h in range(H):
                        w_t = d_pw.tile(
                            [P, 512], BF16, name=f"pw{h}_{cb}", tag=f"pw{h}", bufs=2
                        )
                        eng = nc.gpsimd if (cb >= 2 and h % 2) else nc.sync
                        eng.dma_start(
                            out=w_t[:], in_=projwT[h * P:(h + 1) * P, csl]
                        )
                        pwt.append(w_t)
                    return pwt

                ssp = [[None] * 4 for _ in range(N_RT)]
                pw_next = pwt0
                x_re = []
                for i in range(N_RT):
                    x_t = d_xr.tile([P, C], F32, name=f"xr{i}")
                    eng = nc.scalar if i % 2 else nc.sync
                    eng.dma_start(out=x_t[:], in_=x_loc[i * P:(i + 1) * P, :])
                    x_re.append(x_t)
                for cb in range(4):
                    csl = slice(cb * 512, (cb + 1) * 512)
                    pwt = pw_next
                    if cb < 3:
                        pw_next = pw_loads(cb + 1)
                    for rt in range(N_RT):
                        rsl = slice(rt * P, (rt + 1) * P)
                        ps = d_ps.tile([P, 512], F32, name=f"hp{rt}_{cb}", tag="hp")
                        for h in range(H):
                            nc.tensor.matmul(
                                ps[:],
                                yT[h][:, rsl],
                                pwt[h][:],
                                start=(h == 0),
                                stop=(h == H - 1),
                            )
                        nc.vector.tensor_add(x2[rt][:, csl], x_re[rt][:, csl], ps[:])
                        sqp = d_nb.tile(
                            [P, 512], F32, name=f"dsq{rt}_{cb}", tag="dsq", bufs=3
                        )
                        ssp[rt][cb] = d_st.tile(
                            [P, 1], F32, name=f"dssp{rt}_{cb}", tag=f"ssp{rt}_{cb}"
                        )
                        nc.scalar.activation(
                            sqp[:], x2[rt][:, csl], AF.Square,
                            accum_out=ssp[rt][cb][:],
                        )
                for rt in range(N_RT):
                    s01 = d_st.tile([P, 1], F32, name=f"s01_{rt}", tag="s01", bufs=2)
                    nc.vector.tensor_add(s01[:], ssp[rt][0][:], ssp[rt][1][:])
                    s23 = d_st.tile([P, 1], F32, name=f"s23_{rt}", tag="s23", bufs=2)
                    nc.vector.tensor_add(s23[:], ssp[rt][2][:], ssp[rt][3][:])
                    ss = d_st.tile([P, 1], F32, name=f"dss{rt}", tag="dss", bufs=2)
                    nc.vector.tensor_add(ss[:], s01[:], s23[:])
                    rms = d_st.tile([P, 1], F32, name=f"drms{rt}", tag="drms", bufs=2)
                    nc.scalar.activation(rms[:], ss[:], AF.Sqrt, bias=eps_t[:], scale=1.0 / C)
                    inv = d_st.tile([P, 1], F32, name=f"dinv{rt}", tag="dinv", bufs=2)
                    nc.vector.reciprocal(inv[:], rms[:])
                    nb = d_nb2.tile([P, C], BF16, name=f"n2b{rt}", tag="n2b", bufs=2)
                    nc.scalar.activation(nb[:], x2[rt][:], AF.Copy, scale=inv[:])
                    for j in range(N_KC):
                        tps = d_tps.tile([P, P], BF16, name=f"dtr{rt}_{j}", tag="dtr")
                        nc.tensor.transpose(tps[:], nb[:, j * P:(j + 1) * P], ident_bf[:])
                        nc.scalar.copy(n2T[j][:, rt * P:(rt + 1) * P], tps[:])

            # ---------------- phase E: fc + gelu -> gT ----------------
            g_ctx = ExitStack()
            g_pool = g_ctx.enter_context(tc.tile_pool(name="gT", bufs=1))
            gT = [g_pool.tile([P, R_LOC], BF16, name=f"g{f}") for f in range(N_FT)]
            with (
                tc.tile_pool(name="e_fw", bufs=1) as e_fw,
                tc.tile_pool(name="e_ps", bufs=3, space="PSUM") as e_ps,
            ):
                for fg in range(16):
                    fw = []
                    for ct in range(N_KC):
                        w_t = e_fw.tile(
                            [P, 512], BF16, name=f"fw{ct}_{fg}", tag=f"fw{ct}", bufs=2
                        )
                        eng = nc.gpsimd if (fg > 0 and ct % 2) else nc.sync
                        eng.dma_start(
                            out=w_t[:],
                            in_=fcwT[ct * P:(ct + 1) * P, fg * 512:(fg + 1) * 512],
                        )
                        fw.append(w_t)
                    for ft in range(4):
                        fi = fg * 4 + ft
                        ps = e_ps.tile([P, R_LOC], F32, name=f"gp{fi}", tag="gp")
                        for ct in range(N_KC):
                            nc.tensor.matmul(
                                ps[:],
                                fw[ct][:, ft * P:(ft + 1) * P],
                                n2T[ct][:],
                                start=(ct == 0),
                                stop=(ct == N_KC - 1),
                            )
                        nc.scalar.activation(gT[fi][:], ps[:], AF.Gelu)

            # ---------------- phase F: mlp_proj + residual -> out ----------
            with (
                tc.tile_pool(name="f_mw", bufs=1) as f_mw,
                tc.tile_pool(name="f_ps", bufs=1, space="PSUM") as f_ps,
                tc.tile_pool(name="f_out", bufs=2) as f_out,
            ):
                for cbh in range(2):
                    csl = slice(cbh * 1024, (cbh + 1) * 1024)
                    pss = [
                        f_ps.tile(
                            [P, 1024], F32, name=f"op{cbh}_{rt}", tag=f"op{rt}", bufs=1
                        )
                        for rt in range(N_RT)
                    ]
                    for fi in range(N_FT):
                        mw_t = f_mw.tile(
                            [P, 1024], BF16, name=f"mw{fi}_{cbh}",
                            tag=f"mw{fi % 12}", bufs=2
                        )
                        eng = nc.gpsimd if fi % 2 else nc.sync
                        eng.dma_start(
                            out=mw_t[:], in_=mlpw[fi * P:(fi + 1) * P, csl]
                        )
                        for rt in range(N_RT):
                            for sub in range(2):
                                ssl = slice(sub * 512, (sub + 1) * 512)
                                nc.tensor.matmul(
                                    pss[rt][:, ssl],
                                    gT[fi][:, rt * P:(rt + 1) * P],
                                    mw_t[:, ssl],
                                    start=(fi == 0),
                                    stop=(fi == N_FT - 1),
                                )
                    for rt in range(N_RT):
                        o_t = f_out.tile([P, 1024], F32, name=f"o{cbh}_{rt}", tag="o")
                        nc.vector.tensor_add(o_t[:], x2[rt][:, csl], pss[rt][:])
                        nc.sync.dma_start(
                            out=out_loc[rt * P:(rt + 1) * P, csl], in_=o_t[:]
                        )
            g_ctx.close()
            df_ctx.close()
            yT_ctx.close()

    return nc


_NC_CACHE = None


def _get_nc():
    global _NC_CACHE
    if _NC_CACHE is None:
        _NC_CACHE = build_nc()
    return _NC_CACHE


def _prep_inputs(x, cos, sin, attention_bias, norm1_w, norm2_w, attn_w, proj_w,
                 fc_w, mlp_proj_w):
    bf = ml_dtypes.bfloat16
    xf = np.ascontiguousarray(np.asarray(x, np.float32).reshape(R, C))
    cosTf = np.asarray(cos, np.float32).T.astype(bf)  # [HF, T]
    sinTf = np.asarray(sin, np.float32).T.astype(bf)
    # mask[s, t] = 1 iff s <= t  (transposed causal tril)
    maskT = np.triu(np.ones((P, P), np.float32))
    w1 = np.asarray(norm1_w, np.float32)
    w2 = np.asarray(norm2_w, np.float32)
    aw = (np.asarray(attn_w, np.float32) * w1[None, :]).reshape(H, 3, HD, C)
    # column order [k0 v0 k1 v1 ... | q heads]; qkv comp order is (q,k,v)
    kv_inter = aw[:, (1, 2)].reshape(2 * H, HD, C)  # k_h, v_h interleaved
    aw_kvq = np.concatenate([kv_inter, aw[:, 0]], axis=0)  # [3H, HD, C]
    f8 = ml_dtypes.float8_e4m3fn
    attn_wTn = (aw_kvq.reshape(3 * C, C).T * np.float32(64.0)).astype(f8)
    # [kq, half, p, og2, col] -> [kq, p, og2, half, col]
    attn_w8n = np.ascontiguousarray(
        attn_wTn.reshape(8, 2, P, 6, 1024).transpose(0, 2, 3, 1, 4).reshape(
            8, P, 12288
        )
    )
    projwTn = np.ascontiguousarray(np.asarray(proj_w, np.float32).T).astype(bf)
    fcwTn = np.ascontiguousarray(
        (np.asarray(fc_w, np.float32) * w2[None, :]).T
    ).astype(bf)
    mlpwn = np.ascontiguousarray(np.asarray(mlp_proj_w, np.float32).T).astype(bf)

    in_maps = []
    for c in range(N_CORES):
        t0 = (c * R_LOC) % T
        pm = np.zeros((P, GROUP), np.float32)
        pm[:, : c % GROUP] = 1.0
        in_maps.append({
            "x_loc": np.ascontiguousarray(xf[R_LOC * c:R_LOC * (c + 1)]),
            "cosT": np.ascontiguousarray(cosTf[:, t0:t0 + R_LOC]),
            "sinT": np.ascontiguousarray(sinTf[:, t0:t0 + R_LOC]),
            "maskT": maskT,
            "pmaskp": pm,
            "attn_w8": attn_w8n,
            "projwT": projwTn,
            "fcwT": fcwTn,
            "mlpw": mlpwn,
        })
    return in_maps


def kernel(**inputs):
    nc = _get_nc()
    in_maps = _prep_inputs(**inputs)
    res = run_bass_kernel_spmd(nc, in_maps, list(range(N_CORES)))
    out = np.concatenate(
        [np.asarray(res.results[c]["out_loc"], np.float32) for c in range(N_CORES)],
        axis=0,
    )
    return out.reshape(B, T, C)


# revision 32
# speedup vs baseline: 1.0454x; 1.0143x over previous
"""Trainium2 Bass kernel for nn_Block_42460046688864 (dense transformer block).

Reference math (B=2, T=2048, C=2048, H=16, HD=128):
    n1  = rmsnorm(x) * norm1_w
    qkv = n1 @ attn_w.T ; q,k,v per head ; q,k = rope(q,k) ; phi = elu(.)+1
    w   = (phi_q . phi_k) * scale * tril ; w /= sum(w) ; y = w @ v
    h   = y @ proj_w.T ; x2 = x + h
    ffn = gelu(rmsnorm(x2)*norm2_w @ fc_w.T) @ mlp_proj_w.T ; out = x2 + ffn

Distribution (8 NeuronCores, one NEFF): pure data-parallel over rows.
Each core owns 512 consecutive flattened rows (b-major), computes the
whole block for them, and streams the full weights from HBM (~100MB,
overlapped with ~680us of bf16 matmul).  The causal sum-normalized
elu-kernel attention is computed as chunked linear attention (exactly
equal: the tril mask + positive feature map make masked sum-normalized
scores a prefix recursion; 1/sqrt(HD) and the 1e-8 eps cancel).  The
only cross-core exchange is each core's per-head prefix state
(phi_k^T @ [v|1], 16 x [128,129] bf16 = 528KB), AllGather'd within the
4-core group that shares a batch element, then prefix-masked per core.

Everything else is local: no activation AllGathers/ReduceScatters.

Notes:
  - norm weights are folded into attn_w / fc_w on the host (exact algebra).
  - matmul operands bf16 (fp32 PSUM accumulation); rope/elu elementwise
    runs in bf16 SBUF for the DVE fast modes; residuals stay fp32.
  - TileContext's tail drain is patched to split its semaphore waits:
    this walrus build rejects >2 sync waits on one TPB_CTRL instruction.
"""

from contextlib import ExitStack

import numpy as np
import ml_dtypes

import concourse.bass as bass
import concourse.mybir as mybir
import concourse.tile as tile
from concourse.bass_utils import run_bass_kernel_spmd
from concourse.masks import make_identity
from bass_rust import ScopedClock

F32 = mybir.dt.float32
FP8 = mybir.dt.float8e4
W8SCALE = 64.0
BF16 = mybir.dt.bfloat16
AF = mybir.ActivationFunctionType

N_CORES = 8
GROUP = 4                  # cores per batch element
B, T, C, H, HD = 2, 2048, 2048, 16, 128
HF = HD // 2
R = B * T                  # 4096 flattened rows (b-major)
R_LOC = R // N_CORES       # 512 rows per core
P = 128
N_RT = R_LOC // P          # 4 local row tiles == 4 causal chunks
N_KC = C // P              # 16 contraction tiles over C
FD = 4 * C                 # 8192 mlp hidden
N_FT = FD // P             # 64 hidden tiles
SB = HD + 1                # state cols: [v | 1]
EPS_NORM = 1e-5

_MAX_WAITS = 1  # this walrus build rejects multi-wait instructions


def _split_excess_waits(nc):
    """Move excess semaphore waits onto same-engine NoOps ahead of the op."""
    for fn in nc.m.functions:
        for bb in fn.blocks:
            insts = list(bb.instructions)
            out = []
            for ins in insts:
                si = getattr(ins, "sync_info", None)
                waits = list(si.on_wait) if si and si.on_wait else []
                sem_waits = [w for w in waits if w.sync_type == "semaphore"]
                if len(sem_waits) > _MAX_WAITS:
                    keep = [w for w in waits if w.sync_type != "semaphore"]
                    keep += sem_waits[: _MAX_WAITS - 1] if _MAX_WAITS > 1 else []
                    extra = sem_waits[_MAX_WAITS - 1:] if _MAX_WAITS > 1 else sem_waits
                    for j in range(0, len(extra), _MAX_WAITS):
                        chunk = extra[j:j + _MAX_WAITS]
                        nop = mybir.InstNoOp(
                            name=nc.get_next_instruction_name(), ins=[], outs=[]
                        )
                        nop.engine = ins.engine
                        nop.sync_info = mybir.SyncInfo(on_wait=chunk, on_update=[])
                        out.append(nop)
                    si.on_wait[:] = keep
                out.append(ins)
            if len(out) != len(insts):
                bb.instructions[:] = out


class _TC(tile.TileContext):
    """TileContext whose tail drain splits sem waits one-per-NOP."""

    def schedule_and_allocate(self):
        ret = super().schedule_and_allocate()
        _split_excess_waits(self.nc)
        return ret

    def _drain_and_barrier(self, tick_clock, wait_clock):
        probe = self.nc.sync.nop(nofuse=True, hint="drain_waits")
        wait_clock.add_sem_waits(
            probe.ins, ScopedClock({None: tick_clock.global_clock})
        )
        si = probe.ins.sync_info
        waits = list(si.on_wait) if si and si.on_wait else []
        if len(waits) > 1:
            si.on_wait[:] = waits[:1]
            for w in waits[1:]:
                extra = self.nc.sync.nop(nofuse=True, hint="drain_waits")
                extra.ins.sync_info = mybir.SyncInfo(on_wait=[w], on_update=[])
        self.nc.sync.drain()
        self.nc.all_engine_barrier()
        popped = self.nc._tile_sem_poison_stack.pop()
        assert popped is self._sem_poison
        self.nc.clear_and_free_semaphores(list(self.sems.allocated().values()))
        self.nc.all_engine_barrier()


def build_nc():
    nc = bass.Bass(target_bir_lowering=False)

    x_loc = nc.declare_dram_parameter("x_loc", [R_LOC, C], F32, isOutput=False)
    cosT = nc.declare_dram_parameter("cosT", [HF, R_LOC], BF16, isOutput=False)
    sinT = nc.declare_dram_parameter("sinT", [HF, R_LOC], BF16, isOutput=False)
    maskT = nc.declare_dram_parameter("maskT", [P, P], F32, isOutput=False)
    pmaskp = nc.declare_dram_parameter("pmaskp", [P, GROUP], F32, isOutput=False)
    # attn weight, norm1 folded, transposed; column order [k(16h) | v(16h) | q(16h)]
    attn_w8 = nc.declare_dram_parameter("attn_w8", [N_KC // 2, P, 2 * 3 * C // 1], FP8, isOutput=False)
    projwT = nc.declare_dram_parameter("projwT", [C, C], BF16, isOutput=False)
    fcwT = nc.declare_dram_parameter("fcwT", [C, FD], BF16, isOutput=False)
    mlpw = nc.declare_dram_parameter("mlpw", [FD, C], BF16, isOutput=False)
    out_loc = nc.declare_dram_parameter("out_loc", [R_LOC, C], F32, isOutput=True)

    HH = H // 2
    st_loc = [nc.dram_tensor(f"st_loc{k}", [P, HH * SB], BF16) for k in range(2)]
    st_all = [nc.dram_tensor(f"st_all{k}", [GROUP, P, HH * SB], BF16) for k in range(2)]
    groups = [list(range(GROUP)), list(range(GROUP, 2 * GROUP))]

    with _TC(nc) as tc:
        with ExitStack() as top:
            const = top.enter_context(tc.tile_pool(name="const", bufs=1))
            ident_bf = const.tile([P, P], BF16)
            mask_sb = const.tile([P, P], F32)
            pmask_sb = const.tile([P, GROUP], F32)
            eps_t = const.tile([P, 1], F32)
            cos_sb = const.tile([HF, R_LOC], BF16)
            sin_sb = const.tile([HF, R_LOC], BF16)

            # -------- residents spanning phases A..D (yT) and A..C ---------
            yT_ctx = ExitStack()
            yT_pool = yT_ctx.enter_context(tc.tile_pool(name="yT", bufs=1))
            yT = [yT_pool.tile([P, R_LOC], BF16, name=f"yT{h}") for h in range(H)]

            bc_ctx = ExitStack()
            n1T_pool = bc_ctx.enter_context(tc.tile_pool(name="n1T", bufs=1))
            qkv_pool = bc_ctx.enter_context(tc.tile_pool(name="qkvT", bufs=1))
            vp_pool = bc_ctx.enter_context(tc.tile_pool(name="vp", bufs=1))
            e_pool = bc_ctx.enter_context(tc.tile_pool(name="estate", bufs=1))
            pfx_pool = bc_ctx.enter_context(tc.tile_pool(name="prefix", bufs=1))

            n1T = [
                n1T_pool.tile([P, 2, R_LOC], FP8, name=f"n1T{c}")
                for c in range(N_KC // 2)
            ]
            kTt = [qkv_pool.tile([P, R_LOC], BF16, name=f"kT{h}") for h in range(H)]
            qTt = [qkv_pool.tile([P, R_LOC], BF16, name=f"qT{h}") for h in range(H)]
            vp = [
                [vp_pool.tile([P, SB], BF16, name=f"vp{h}_{i}") for i in range(N_RT)]
                for h in range(H)
            ]
            for h in range(H):
                for i in range(N_RT):
                    nc.vector.memset(vp[h][i][:, HD:SB], 1.0)
            # bf16 exclusive local-state snapshots E_1..E_3 per head + f32 chain
            e_st = [
                [e_pool.tile([P, SB], BF16, name=f"e{h}_{i}") for i in range(3)]
                for h in range(H)
            ]

            # ---------------- phase A: load x, rmsnorm, transpose -> n1T ----
            a_ctx = ExitStack()
            xa_pool = a_ctx.enter_context(tc.tile_pool(name="xa", bufs=1))
            x_tiles = []
            for i in range(N_RT):
                x_t = xa_pool.tile([P, C], F32, name=f"xa{i}")
                eng = nc.scalar if i % 2 else nc.sync
                eng.dma_start(out=x_t[:], in_=x_loc[i * P:(i + 1) * P, :])
                x_tiles.append(x_t)
            make_identity(nc, ident_bf)
            nc.sync.dma_start(out=mask_sb[:], in_=maskT[:, :])
            nc.sync.dma_start(out=pmask_sb[:], in_=pmaskp[:, :])
            nc.vector.memset(eps_t[:], EPS_NORM)
            nc.sync.dma_start(out=cos_sb[:], in_=cosT[:, :])
            nc.sync.dma_start(out=sin_sb[:], in_=sinT[:, :])
            with (
                tc.tile_pool(name="a_st", bufs=1) as a_st,
                tc.tile_pool(name="a_nb", bufs=1) as a_nb,
                tc.tile_pool(name="a_ps", bufs=4, space="PSUM") as a_ps,
            ):
                for i in range(N_RT):
                    x_t = x_tiles[i]
                    sq = a_nb.tile([P, C], F32, name=f"sq{i}", tag="sq", bufs=2)
                    ss = a_st.tile([P, 1], F32, name=f"ss{i}", tag="ss", bufs=2)
                    nc.scalar.activation(sq[:], x_t[:], AF.Square, accum_out=ss[:])
                    rms = a_st.tile([P, 1], F32, name=f"rms{i}", tag="rms", bufs=2)
                    nc.scalar.activation(rms[:], ss[:], AF.Sqrt, bias=eps_t[:], scale=1.0 / C)
                    inv = a_st.tile([P, 1], F32, name=f"inv{i}", tag="inv", bufs=2)
                    nc.vector.reciprocal(inv[:], rms[:])
                    nb = a_nb.tile([P, C], BF16, name=f"n1b{i}", tag="n1b", bufs=2)
                    nc.scalar.activation(nb[:], x_t[:], AF.Copy, scale=inv[:])
                    for j in range(N_KC):
                        ps = a_ps.tile([P, P], BF16, name=f"atr{i}_{j}", tag="atr")
                        nc.tensor.transpose(ps[:], nb[:, j * P:(j + 1) * P], ident_bf[:])
                        nc.scalar.copy(
                            n1T[j // 2][:, j % 2, i * P:(i + 1) * P], ps[:]
                        )
            a_ctx.close()

            # ---------------- phase B: qkv + rope/elu + states + AllGather --
            def rope_elu(ps, dst, rp):
                """psum [P,512] (hd x t) -> dst bf16 [P,512] = elu(rope(.))+1.

                The two psum halves are evicted into separate base-0 tiles:
                DVE tensor-tensor requires equal base partitions for SBUF
                operands, and base-0 keeps the all-bf16 2x path legal.
                """
                qe1 = rp.tile([HF, R_LOC], BF16, name="qe1", tag="qe1", bufs=3)
                nc.scalar.activation(qe1[:], ps[0:HF, :], AF.Copy, scale=1.0 / W8SCALE)
                qe2 = rp.tile([HF, R_LOC], BF16, name="qe2", tag="qe2", bufs=3)
                nc.scalar.activation(qe2[:], ps[HF:P, :], AF.Copy, scale=1.0 / W8SCALE)
                ro = rp.tile([P, R_LOC], BF16, name="ro", tag="ro", bufs=2)
                s1 = rp.tile([HF, R_LOC], BF16, name="s1", tag="s1", bufs=2)
                s2 = rp.tile([HF, R_LOC], BF16, name="s2", tag="s2", bufs=2)
                nc.vector.tensor_mul(s1[:], qe1[:], cos_sb[:])
                nc.vector.tensor_mul(s2[:], qe2[:], sin_sb[:])
                nc.vector.tensor_sub(ro[0:HF, :], s1[:], s2[:])
                s3 = rp.tile([HF, R_LOC], BF16, name="s3", tag="s3", bufs=2)
                s4 = rp.tile([HF, R_LOC], BF16, name="s4", tag="s4", bufs=2)
                nc.vector.tensor_mul(s3[:], qe1[:], sin_sb[:])
                nc.vector.tensor_mul(s4[:], qe2[:], cos_sb[:])
                nc.vector.tensor_add(ro[HF:P, :], s3[:], s4[:])
                # phi = relu(ro) + exp(min(ro, 0))
                rl = rp.tile([P, R_LOC], BF16, name="rl", tag="rl", bufs=2)
                nc.vector.tensor_scalar_max(rl[:], ro[:], 0.0)
                dm = rp.tile([P, R_LOC], BF16, name="dm", tag="dm", bufs=2)
                nc.vector.tensor_scalar_min(dm[:], ro[:], 0.0)
                ex = rp.tile([P, R_LOC], BF16, name="ex", tag="ex", bufs=2)
                nc.scalar.activation(ex[:], dm[:], AF.Exp)
                nc.vector.tensor_add(dst[:], rl[:], ex[:])

            with (
                tc.tile_pool(name="b_aw", bufs=1) as b_aw,
                tc.tile_pool(name="b_rp", bufs=1) as b_rp,
                tc.tile_pool(name="b_vt", bufs=4) as b_vt,
                tc.tile_pool(name="b_kp", bufs=1) as b_kp,
                tc.tile_pool(name="b_stb", bufs=4) as b_stb,
                tc.tile_pool(name="b_ps", bufs=3, space="PSUM") as b_ps,
                tc.tile_pool(name="b_sd", bufs=3, space="PSUM") as b_sd,
                tc.tile_pool(name="b_tr", bufs=2, space="PSUM") as b_tr,
            ):
                kp = [
                    [b_kp.tile([P, P], BF16, name=f"kp{h}_{i}") for i in range(N_RT)]
                    for h in range(H)
                ]

                def state_chain(h):
                    # prefix sums as independent PSUM accumulation groups: no
                    # cross-engine serial chain, just 10 tiny matmuls
                    for n in range(N_RT):  # E_{n+1} = sum_{i<=n} Sd_i
                        sd = b_sd.tile([P, SB], F32, name=f"sd{h}_{n}", tag="sd")
                        for i in range(n + 1):
                            nc.tensor.matmul(
                                sd[:], kp[h][i][:], vp[h][i][:],
                                start=(i == 0), stop=(i == n),
                            )
                        if n < 3:
                            nc.vector.tensor_copy(e_st[h][n][:], sd[:])
                        else:
                            tb = b_stb.tile([P, SB], BF16, name=f"tb{h}", tag="tb")
                            nc.vector.tensor_copy(tb[:], sd[:])
                            nc.scalar.dma_start(
                                out=st_loc[h // HH][
                                    :, (h % HH) * SB:(h % HH + 1) * SB
                                ],
                                in_=tb[:],
                            )

                def qkv_block2(og2):
                    # one contiguous DMA per (k-pair, og-pair): host layout
                    # attn_w8[kq, p, og2*2048 + half*1024 + col]
                    aw = []
                    for kq in range(N_KC // 2):
                        w_t = b_aw.tile(
                            [P, 2, 1024], FP8, name=f"aw{kq}_{og2}", tag=f"aw{kq}",
                            bufs=2,
                        )
                        eng = nc.gpsimd if kq % 2 else nc.sync
                        eng.dma_start(
                            out=w_t[:],
                            in_=attn_w8[kq, :, og2 * 2048:(og2 + 1) * 2048],
                        )
                        aw.append(w_t)
                    for ot in range(8):
                        j = og2 * 8 + ot
                        ps = b_ps.tile([P, R_LOC], F32, name=f"qkvp{j}", tag="qkvp")
                        for kq in range(N_KC // 2):
                            nc.tensor.matmul(
                                ps[:],
                                aw[kq][:, :, ot * P:(ot + 1) * P],
                                n1T[kq][:],
                                start=(kq == 0),
                                stop=(kq == N_KC // 2 - 1),
                                perf_mode=mybir.MatmulPerfMode.DoubleRow,
                            )
                        if og2 < 4:  # interleaved [k_h | v_h] pairs
                            h = j // 2
                            if j % 2 == 0:
                                rope_elu(ps, kTt[h], b_rp)
                                for i in range(N_RT):
                                    ktr = b_tr.tile(
                                        [P, P], BF16, name=f"ktr{h}_{i}", tag="tr"
                                    )
                                    nc.tensor.transpose(
                                        ktr[:], kTt[h][:, i * P:(i + 1) * P],
                                        ident_bf[:],
                                    )
                                    if i % 2:
                                        nc.vector.tensor_copy(kp[h][i][:], ktr[:])
                                    else:
                                        nc.scalar.copy(kp[h][i][:], ktr[:])
                            else:
                                vt = b_vt.tile([P, R_LOC], BF16, name=f"vT{h}", tag="vT")
                                nc.scalar.activation(
                                    vt[:], ps[:], AF.Copy, scale=1.0 / W8SCALE
                                )
                                for i in range(N_RT):
                                    vtr = b_tr.tile(
                                        [P, P], BF16, name=f"vtr{h}_{i}", tag="tr"
                                    )
                                    nc.tensor.transpose(
                                        vtr[:], vt[:, i * P:(i + 1) * P], ident_bf[:]
                                    )
                                    if i % 2:
                                        nc.vector.tensor_copy(vp[h][i][:, 0:HD], vtr[:])
                                    else:
                                        nc.scalar.copy(vp[h][i][:, 0:HD], vtr[:])
                        else:  # q head j-32
                            h = j - 2 * H
                            rope_elu(ps, qTt[h], b_rp)

                # interleaved k/v column groups; each head's state chain is
                # emitted one og later so PE never waits on the transposes.
                # The state gather is split in half so the first collective
                # flies while the second half of k/v is still computing.
                def gather(k):
                    nc.gpsimd.collective_compute(
                        "AllGather",
                        mybir.AluOpType.bypass,
                        ins=[st_loc[k].ap().opt()],
                        outs=[st_all[k].ap().opt()],
                        replica_groups=groups,
                    )

                for og2 in range(4):
                    qkv_block2(og2)
                    for hh in range(4 * (og2 - 1), 4 * og2):
                        if hh >= 0:
                            state_chain(hh)
                    if og2 == 2:
                        gather(0)
                for hh in range(12, 16):
                    state_chain(hh)
                gather(1)

                # q column groups (overlap the collective transfers)
                for og2 in range(4, 6):
                    qkv_block2(og2)

            # ---------------- phase C: prefix + attention ----------------
            with (
                tc.tile_pool(name="c_g", bufs=1) as c_g,
                tc.tile_pool(name="c_cmb", bufs=8) as c_cmb,
                tc.tile_pool(name="c_am", bufs=4) as c_am,
                tc.tile_pool(name="c_sm", bufs=1) as c_sm,
                tc.tile_pool(name="c_aps", bufs=2, space="PSUM") as c_aps,
                tc.tile_pool(name="c_yps", bufs=3, space="PSUM") as c_yps,
                tc.tile_pool(name="c_ytr", bufs=2, space="PSUM") as c_ytr,
            ):
                pwt0 = []
                for h in range(H):
                    w_t = yT_pool.tile([P, 512], BF16, name=f"pw0_{h}")
                    eng = nc.gpsimd if h % 2 else nc.sync
                    eng.dma_start(out=w_t[:], in_=projwT[h * P:(h + 1) * P, 0:512])
                    pwt0.append(w_t)
                pfx = [
                    pfx_pool.tile([P, HH * SB], BF16, name=f"prefix{k}")
                    for k in range(2)
                ]
                gtmp = [
                    pfx_pool.tile([P, HH * SB], BF16, name=f"gtmp{k}")
                    for k in range(2)
                ]
                for k in range(2):
                    for j in range(GROUP):
                        g_t = c_g.tile(
                            [P, HH * SB], BF16, name=f"g{k}_{j}", tag="g", bufs=3
                        )
                        nc.sync.dma_start(out=g_t[:], in_=st_all[k][j])
                        if j == 0:
                            nc.vector.tensor_scalar_mul(
                                pfx[k][:], g_t[:], pmask_sb[:, 0:1]
                            )
                        else:
                            nc.vector.tensor_scalar_mul(
                                gtmp[k][:], g_t[:], pmask_sb[:, j:j + 1]
                            )
                            nc.vector.tensor_add(
                                pfx[k][:], pfx[k][:], gtmp[k][:]
                            )

                for h in range(H):
                    pfx_h = pfx[h // HH]
                    hsl = slice((h % HH) * SB, (h % HH + 1) * SB)
                    for i in range(N_RT):
                        isl = slice(i * P, (i + 1) * P)
                        cmb = c_cmb.tile([P, SB], BF16, name=f"cmb{h}_{i}", tag="cmb")
                        if i == 0:
                            nc.gpsimd.tensor_copy(cmb[:], pfx_h[:, hsl])
                        else:
                            nc.vector.tensor_add(
                                cmb[:], pfx_h[:, hsl], e_st[h][i - 1][:]
                            )
                        a_ps = c_aps.tile([P, P], F32, name=f"a{h}_{i}", tag="a")
                        nc.tensor.matmul(
                            a_ps[:], kTt[h][:, isl], qTt[h][:, isl],
                            start=True, stop=True,
                        )
                        am = c_am.tile([P, P], BF16, name=f"am{h}_{i}", tag="am")
                        nc.vector.tensor_mul(am[:], a_ps[:], mask_sb[:])
                        y_ps = c_yps.tile([P, SB], F32, name=f"y{h}_{i}", tag="y")
                        nc.tensor.matmul(
                            y_ps[:], qTt[h][:, isl], cmb[:], start=True, stop=False
                        )
                        nc.tensor.matmul(
                            y_ps[:], am[:], vp[h][i][:], start=False, stop=True
                        )
                        rec = c_sm.tile([P, 1], F32, name=f"rec{h}_{i}", tag="rec", bufs=4)
                        nc.vector.reciprocal(rec[:], y_ps[:, HD:SB])
                        y_bf = c_sm.tile([P, HD], BF16, name=f"yb{h}_{i}", tag="yb", bufs=4)
                        nc.scalar.activation(y_bf[:], y_ps[:, 0:HD], AF.Copy, scale=rec[:])
                        ytr = c_ytr.tile([P, P], BF16, name=f"ytr{h}_{i}", tag="ytr")
                        nc.tensor.transpose(ytr[:], y_bf[:], ident_bf[:])
                        if h % 2:
                            nc.vector.tensor_copy(yT[h][:, isl], ytr[:])
                        else:
                            nc.scalar.copy(yT[h][:, isl], ytr[:])

            bc_ctx.close()  # free qkv residents for the MLP phases

            # ---------------- phase D: proj + residual + rmsnorm2 ----------
            # x2 spans D..F; n2T and gT span D..F region
            df_ctx = ExitStack()
            x2_pool = df_ctx.enter_context(tc.tile_pool(name="x2", bufs=1))
            n2T_pool = df_ctx.enter_context(tc.tile_pool(name="n2T", bufs=1))
            x2 = [x2_pool.tile([P, C], F32, name=f"x2_{i}") for i in range(N_RT)]
            n2T = [n2T_pool.tile([P, R_LOC], BF16, name=f"n2T{c}") for c in range(N_KC)]

            with (
                tc.tile_pool(name="d_pw", bufs=1) as d_pw,
                tc.tile_pool(name="d_xr", bufs=1) as d_xr,
                tc.tile_pool(name="d_st", bufs=1) as d_st,
                tc.tile_pool(name="d_nb", bufs=1) as d_nb,
                tc.tile_pool(name="d_nb2", bufs=1) as d_nb2,
                tc.tile_pool(name="d_ps", bufs=3, space="PSUM") as d_ps,
                tc.tile_pool(name="d_tps", bufs=4, space="PSUM") as d_tps,
            ):
                def pw_loads(cb):
                    csl = slice(cb * 512, (cb + 1) * 512)
                    pwt = []
                    for h in range(H):
                        w_t = d_pw.tile(
                            [P, 512], BF16, name=f"pw{h}_{cb}", tag=f"pw{h}", bufs=2
                        )
                        eng = nc.gpsimd if (cb >= 2 and h % 2) else nc.sync
                        eng.dma_start(
                            out=w_t[:], in_=projwT[h * P:(h + 1) * P, csl]
                        )
                        pwt.append(w_t)
                    return pwt

                ssp = [[None] * 4 for _ in range(N_RT)]
                pw_next = pwt0
                x_re = []
                for i in range(N_RT):
                    x_t = d_xr.tile([P, C], F32, name=f"xr{i}")
                    eng = nc.scalar if i % 2 else nc.sync
                    eng.dma_start(out=x_t[:], in_=x_loc[i * P:(i + 1) * P, :])
                    x_re.append(x_t)
                for cb in range(4):
                    csl = slice(cb * 512, (cb + 1) * 512)
                    pwt = pw_next
                    if cb < 3:
                        pw_next = pw_loads(cb + 1)
                    for rt in range(N_RT):
                        rsl = slice(rt * P, (rt + 1) * P)
                        ps = d_ps.tile([P, 512], F32, name=f"hp{rt}_{cb}", tag="hp")
                        for h in range(H):
                            nc.tensor.matmul(
                                ps[:],
                                yT[h][:, rsl],
                                pwt[h][:],
                                start=(h == 0),
                                stop=(h == H - 1),
                            )
                        nc.vector.tensor_add(x2[rt][:, csl], x_re[rt][:, csl], ps[:])
                        sqp = d_nb.tile(
                            [P, 512], F32, name=f"dsq{rt}_{cb}", tag="dsq", bufs=3
                        )
                        ssp[rt][cb] = d_st.tile(
                            [P, 1], F32, name=f"dssp{rt}_{cb}", tag=f"ssp{rt}_{cb}"
                        )
                        nc.scalar.activation(
                            sqp[:], x2[rt][:, csl], AF.Square,
                            accum_out=ssp[rt][cb][:],
                        )
                for rt in range(N_RT):
                    s01 = d_st.tile([P, 1], F32, name=f"s01_{rt}", tag="s01", bufs=2)
                    nc.vector.tensor_add(s01[:], ssp[rt][0][:], ssp[rt][1][:])
                    s23 = d_st.tile([P, 1], F32, name=f"s23_{rt}", tag="s23", bufs=2)
                    nc.vector.tensor_add(s23[:], ssp[rt][2][:], ssp[rt][3][:])
                    ss = d_st.tile([P, 1], F32, name=f"dss{rt}", tag="dss", bufs=2)
                    nc.vector.tensor_add(ss[:], s01[:], s23[:])
                    rms = d_st.tile([P, 1], F32, name=f"drms{rt}", tag="drms", bufs=2)
                    nc.scalar.activation(rms[:], ss[:], AF.Sqrt, bias=eps_t[:], scale=1.0 / C)
                    inv = d_st.tile([P, 1], F32, name=f"dinv{rt}", tag="dinv", bufs=2)
                    nc.vector.reciprocal(inv[:], rms[:])
                    nb = d_nb2.tile([P, C], BF16, name=f"n2b{rt}", tag="n2b", bufs=2)
                    nc.scalar.activation(nb[:], x2[rt][:], AF.Copy, scale=inv[:])
                    for j in range(N_KC):
                        tps = d_tps.tile([P, P], BF16, name=f"dtr{rt}_{j}", tag="dtr")
                        nc.tensor.transpose(tps[:], nb[:, j * P:(j + 1) * P], ident_bf[:])
                        nc.scalar.copy(n2T[j][:, rt * P:(rt + 1) * P], tps[:])

            # ---------------- phase E: fc + gelu -> gT ----------------
            g_ctx = ExitStack()
            g_pool = g_ctx.enter_context(tc.tile_pool(name="gT", bufs=1))
            gT = [g_pool.tile([P, R_LOC], BF16, name=f"g{f}") for f in range(N_FT)]
            with (
                tc.tile_pool(name="e_fw", bufs=1) as e_fw,
                tc.tile_pool(name="e_ps", bufs=3, space="PSUM") as e_ps,
            ):
                for fg in range(16):
                    fw = []
                    for ct in range(N_KC):
                        w_t = e_fw.tile(
                            [P, 512], BF16, name=f"fw{ct}_{fg}", tag=f"fw{ct}", bufs=2
                        )
                        eng = nc.gpsimd if (fg > 0 and ct % 2) else nc.sync
                        eng.dma_start(
                            out=w_t[:],
                            in_=fcwT[ct * P:(ct + 1) * P, fg * 512:(fg + 1) * 512],
                        )
                        fw.append(w_t)
                    for ft in range(4):
                        fi = fg * 4 + ft
                        ps = e_ps.tile([P, R_LOC], F32, name=f"gp{fi}", tag="gp")
                        for ct in range(N_KC):
                            nc.tensor.matmul(
                                ps[:],
                                fw[ct][:, ft * P:(ft + 1) * P],
                                n2T[ct][:],
                                start=(ct == 0),
                                stop=(ct == N_KC - 1),
                            )
                        nc.scalar.activation(gT[fi][:], ps[:], AF.Gelu)

            # ---------------- phase F: mlp_proj + residual -> out ----------
            with (
                tc.tile_pool(name="f_mw", bufs=1) as f_mw,
                tc.tile_pool(name="f_ps", bufs=1, space="PSUM") as f_ps,
                tc.tile_pool(name="f_out", bufs=2) as f_out,
            ):
                for cbh in range(2):
                    csl = slice(cbh * 1024, (cbh + 1) * 1024)
                    pss = [
                        f_ps.tile(
                            [P, 1024], F32, name=f"op{cbh}_{rt}", tag=f"op{rt}", bufs=1
                        )
                        for rt in range(N_RT)
                    ]
                    for fi in range(N_FT):
                        mw_t = f_mw.tile(
                            [P, 1024], BF16, name=f"mw{fi}_{cbh}",
                            tag=f"mw{fi % 12}", bufs=2
                        )
                        eng = nc.gpsimd if fi % 2 else nc.sync
                        eng.dma_start(
                            out=mw_t[:], in_=mlpw[fi * P:(fi + 1) * P, csl]
                        )
                        for rt in range(N_RT):
                            for sub in range(2):
                                ssl = slice(sub * 512, (sub + 1) * 512)
                                nc.tensor.matmul(
                                    pss[rt][:, ssl],
                                    gT[fi][:, rt * P:(rt + 1) * P],
                                    mw_t[:, ssl],
                                    start=(fi == 0),
                                    stop=(fi == N_FT - 1),
                                )
                    for rt in range(N_RT):
                        o_t = f_out.tile([P, 1024], F32, name=f"o{cbh}_{rt}", tag="o")
                        nc.vector.tensor_add(o_t[:], x2[rt][:, csl], pss[rt][:])
                        nc.sync.dma_start(
                            out=out_loc[rt * P:(rt + 1) * P, csl], in_=o_t[:]
                        )
            g_ctx.close()
            df_ctx.close()
            yT_ctx.close()

    return nc


_NC_CACHE = None


def _get_nc():
    global _NC_CACHE
    if _NC_CACHE is None:
        _NC_CACHE = build_nc()
    return _NC_CACHE


def _prep_inputs(x, cos, sin, attention_bias, norm1_w, norm2_w, attn_w, proj_w,
                 fc_w, mlp_proj_w):
    bf = ml_dtypes.bfloat16
    xf = np.ascontiguousarray(np.asarray(x, np.float32).reshape(R, C))
    cosTf = np.asarray(cos, np.float32).T.astype(bf)  # [HF, T]
    sinTf = np.asarray(sin, np.float32).T.astype(bf)
    # mask[s, t] = 1 iff s <= t  (transposed causal tril)
    maskT = np.triu(np.ones((P, P), np.float32))
    w1 = np.asarray(norm1_w, np.float32)
    w2 = np.asarray(norm2_w, np.float32)
    aw = (np.asarray(attn_w, np.float32) * w1[None, :]).reshape(H, 3, HD, C)
    # column order [k0 v0 k1 v1 ... | q heads]; qkv comp order is (q,k,v)
    kv_inter = aw[:, (1, 2)].reshape(2 * H, HD, C)  # k_h, v_h interleaved
    aw_kvq = np.concatenate([kv_inter, aw[:, 0]], axis=0)  # [3H, HD, C]
    f8 = ml_dtypes.float8_e4m3fn
    attn_wTn = (aw_kvq.reshape(3 * C, C).T * np.float32(64.0)).astype(f8)
    # [kq, half, p, og2, col] -> [kq, p, og2, half, col]
    attn_w8n = np.ascontiguousarray(
        attn_wTn.reshape(8, 2, P, 6, 1024).transpose(0, 2, 3, 1, 4).reshape(
            8, P, 12288
        )
    )
    projwTn = np.ascontiguousarray(np.asarray(proj_w, np.float32).T).astype(bf)
    fcwTn = np.ascontiguousarray(
        (np.asarray(fc_w, np.float32) * w2[None, :]).T
    ).astype(bf)
    mlpwn = np.ascontiguousarray(np.asarray(mlp_proj_w, np.float32).T).astype(bf)

    in_maps = []
    for c in range(N_CORES):
        t0 = (c * R_LOC) % T
        pm = np.zeros((P, GROUP), np.float32)
        pm[:, : c % GROUP] = 1.0
        in_maps.append({
            "x_loc": np.ascontiguousarray(xf[R_LOC * c:R_LOC * (c + 1)]),
            "cosT": np.ascontiguousarray(cosTf[:, t0:t0 + R_LOC]),
            "sinT": np.ascontiguousarray(sinTf[:, t0:t0 + R_LOC]),
            "maskT": maskT,
            "pmaskp": pm,
            "attn_w8": attn_w8n,
            "projwT": projwTn,
            "fcwT": fcwTn,
            "mlpw": mlpwn,
        })
    return in_maps


def kernel(**inputs):
    nc = _get_nc()
    in_maps = _prep_inputs(**inputs)
    res = run_bass_kernel_spmd(nc, in_maps, list(range(N_CORES)))
    out = np.concatenate(
        [np.asarray(res.results[c]["out_loc"], np.float32) for c in range(N_CORES)],
        axis=0,
    )
    return out.reshape(B, T, C)
